# revision 1
# baseline (speedup 1.0000x reference)
"""Trainium2 Bass kernel for nn_AdaFeatBlock (modulated deformable-conv block).

Sharding: data-parallel over batch — 8 samples -> 8 NeuronCores, all weights
replicated; each core computes its sample end-to-end, host stacks outputs.

Per-core pipeline (one sample, x [64,128,128]):
  1. x -> bf16 "half-split" padded layout: partition h*64+c = channel c of
     image-half h; free = 76 stored rows (h*64-6 .. h*64+69) x 130 cols
     (-1..128), zero-padded borders.
  2. offset/mask 3x3 conv = 9 shifted matmuls, block-diagonal [128, 54]
     lhsT (both halves at once), PSUM-accumulated. Output row order per
     half: [off_y k0..8 | off_x k0..8 | mask k0..8].
  3. Coordinate math on [18, 8192] tiles (partition = (half, k)): bilinear
     corner weights (mask-modulated, zero outside the image via the
     zero-padded gather table) and 2x2-quad table indices.
  4. Quad gather table Q[128, 10032, 4] bf16: 2x2 pixel blocks at all 4
     row/col parities. ap_gather (d=2 f32 view = 8B quad) fetches a
     point's 4 corners for 16 channels/core; 8 Q7 cores cover
     128 partitions = 2 halves x 64 channels.
  5. Per (k, 512-px block): corner-weight rows broadcast to 128 partitions
     via a tiny selector matmul into PSUM; fused PSUM-read multiply into
     gathered corners; 3 adds -> modulated bilinear sample "val".
  6. Deformable einsum: per k a [128->128, 512] matmul with block-diagonal
     channel-duplicated w_dc, PSUM-accumulated over all 9 k. + b_dc -> out.
"""

import numpy as np

import concourse.bass as bass
import concourse.tile as tile
from concourse import mybir
from concourse.bass_utils import run_bass_kernel_spmd
from concourse import library_config
from concourse.library_overlay import lower_extended_insts
from concourse.vector_clock import ScopedClock

AF = mybir.ActivationFunctionType
ALU = mybir.AluOpType
DT = mybir.dt

B, C, H, W = 8, 64, 128, 128
O = 64
K = 3
KF = 9
NCORES = 8
HALF = H // 2
NPIX = H * W // 2              # 8192 pixels per half
ROWS_ST = 76                   # stored rows per half
PITCH = 130                    # stored cols (-1..128)
RY_N, RX_N = 38, 66
NBLK = 4 * RY_N * RX_N         # 10032
GCHUNK = 2048
SUB = 512
S16 = NPIX // 16               # idx ints per partition per k


def _install_compat():
    """This walrus build accepts at most ONE sync-wait per instruction."""
    if getattr(tile.TileContext, "_adafeat_patched", False):
        return
    _orig_lower = tile.TileContext._lower_ordered_insts

    def _split_waits(nc, ordered):
        for insts in ordered.values():
            new_insts = []
            for inst in insts:
                si = inst.sync_info
                if si is not None and si.on_wait and len(si.on_wait) > 1:
                    waits = list(si.on_wait)
                    for w in waits[:-1]:
                        nop = mybir.InstNoOp(name=f"I-{nc.next_id()}", ins=[], outs=[])
                        nop.engine = inst.engine
                        nop.sync_info = mybir.SyncInfo(on_wait=[w], on_update=[])
                        new_insts.append(nop)
                    inst.sync_info = mybir.SyncInfo(
                        on_wait=[waits[-1]], on_update=list(si.on_update)
                    )
                new_insts.append(inst)
            insts[:] = new_insts

    def _lower_split(self, ordered):
        _split_waits(self.nc, ordered)
        return _orig_lower(self, ordered)

    def _drain_split(self, tick_clock, wait_clock):
        carrier = self.nc.sync.nop(nofuse=True)
        wait_clock.add_sem_waits(
            carrier.ins, ScopedClock({None: tick_clock.global_clock})
        )
        si = carrier.ins.sync_info
        if si is not None and si.on_wait and len(si.on_wait) > 1:
            waits = list(si.on_wait)
            carrier.ins.sync_info = mybir.SyncInfo(
                on_wait=waits[:1], on_update=list(si.on_update)
            )
            for w in waits[1:]:
                extra = self.nc.sync.nop(nofuse=True)
                extra.ins.sync_info = mybir.SyncInfo(on_wait=[w], on_update=[])
        self.nc.sync.drain()
        self.nc.all_engine_barrier()
        popped = self.nc._tile_sem_poison_stack.pop()
        assert popped is self._sem_poison
        self.nc.clear_and_free_semaphores(list(self.sems.allocated().values()))
        self.nc.all_engine_barrier()

    tile.TileContext._lower_ordered_insts = _lower_split
    tile.TileContext._drain_and_barrier = _drain_split
    tile.TileContext._adafeat_patched = True


def _emit(nc, tc, x_ext, wom_ext, bom_ext, wdc_ext, bdc_ext, out_ext):
    _iotas = []

    with tc.tile_pool(name="persist", bufs=1) as persist:
        x_sb = persist.tile([128, ROWS_ST * PITCH], DT.bfloat16)
        wq = persist.tile([128, NPIX], DT.bfloat16)
        idxt = persist.tile([128, KF * S16], DT.int16)
        wdup = persist.tile([128, KF * 128], DT.bfloat16)
        sel = persist.tile([128, KF * 4 * 128], DT.bfloat16)
        bdc_t = persist.tile([128, 1], DT.float32)

        x3 = lambda: x_sb[:].rearrange("p (r c) -> p r c", c=PITCH)

        # ======== phase 1: load x (f32 -> bf16), half-split, zero-padded
        nc.vector.memset(x_sb[:], 0.0)
        nc.vector.memset(wq[:], 0.0)
        xv = x_ext[:]
        for h in range(2):
            r0 = max(0, h * HALF - 6)
            r1 = min(H - 1, h * HALF + 69)
            nrow = r1 - r0 + 1
            rloc = r0 - (h * HALF - 6)
            dst = x3()[h * 64 : h * 64 + 64, rloc : rloc + nrow, 1 : 1 + W]
            nc.gpsimd.dma_start(out=dst, in_=xv[:, r0 : r1 + 1, :])

        # ======== phase 2: offset/mask conv
        with (
            tc.tile_pool(name="convw", bufs=1) as convw,
            tc.tile_pool(name="omp", bufs=1) as omp,
            tc.tile_pool(name="convp", bufs=2, space="PSUM") as convp,
        ):
            # w_om views: y/x roles from rows 0..17 (o=2k+r), m role rows 18..26
            wom_yx = wom_ext[:][0:18].rearrange(
                "(o2 r) c kh kw -> c o2 r (kh kw)", r=2
            )
            wom_m = wom_ext[:][18:27].rearrange("o c kh kw -> c o (kh kw)")
            lhs_om = []
            for dy in range(3):
                for dx in range(3):
                    dd = dy * K + dx
                    t = convw.tile([128, 54], DT.bfloat16, tag=f"lom{dd}")
                    nc.vector.memset(t[:], 0.0)
                    for h in range(2):
                        ps = slice(h * 64, h * 64 + 64)
                        nc.gpsimd.dma_start(
                            out=t[ps, h * 27 + 0 : h * 27 + 9],
                            in_=wom_yx[:, 0:9, 0:1, dd : dd + 1].rearrange(
                                "c a b d -> c (a b d)"),
                        )
                        nc.gpsimd.dma_start(
                            out=t[ps, h * 27 + 9 : h * 27 + 18],
                            in_=wom_yx[:, 0:9, 1:2, dd : dd + 1].rearrange(
                                "c a b d -> c (a b d)"),
                        )
                        nc.gpsimd.dma_start(
                            out=t[ps, h * 27 + 18 : h * 27 + 27],
                            in_=wom_m[:, :, dd : dd + 1].rearrange(
                                "c a d -> c (a d)"),
                        )
                    lhs_om.append(t)

            bom_t = convw.tile([54, 1], DT.float32)
            bom_yx = bom_ext[:][0:18].rearrange("(o2 r) -> o2 r", r=2)
            for h in range(2):
                nc.sync.dma_start(
                    out=bom_t[h * 27 + 0 : h * 27 + 9, 0:1], in_=bom_yx[0:9, 0:1]
                )
                nc.sync.dma_start(
                    out=bom_t[h * 27 + 9 : h * 27 + 18, 0:1], in_=bom_yx[0:9, 1:2]
                )
                nc.sync.dma_start(
                    out=bom_t[h * 27 + 18 : h * 27 + 27, 0:1],
                    in_=bom_ext[:][18:27].rearrange("(o one) -> o one", one=1),
                )

            om = omp.tile([54, NPIX], DT.float32)
            rows_per_sub = SUB // W  # 4
            for cb in range(NPIX // SUB):
                pt = convp.tile([54, SUB], DT.float32, tag="cpt")
                r0 = cb * rows_per_sub
                for i, (dy, dx) in enumerate(
                    (dy, dx) for dy in range(3) for dx in range(3)
                ):
                    rhs = x3()[:, 6 + r0 + dy - 1 : 6 + r0 + dy - 1 + rows_per_sub,
                               dx : dx + W]
                    nc.tensor.matmul(
                        out=pt[:], lhsT=lhs_om[i][:], rhs=rhs,
                        start=(i == 0), stop=(i == 8),
                    )
                nc.vector.tensor_scalar(
                    out=om[:, cb * SUB : (cb + 1) * SUB], in0=pt[:],
                    scalar1=bom_t[:, 0:1], scalar2=None, op0=ALU.add,
                )

            # ======== phase 3: coordinate math, chunked, all tiles base-0
            with tc.tile_pool(name="math", bufs=1) as mpool:
                idx16 = mpool.tile([18, NPIX], DT.int16)
                OY = mpool.tile([18, 2048], DT.float32)
                OX = mpool.tile([18, 2048], DT.float32)
                OM = mpool.tile([18, 2048], DT.float32)
                IOT = mpool.tile([18, 2048], DT.float32)
                T0 = mpool.tile([18, 2048], DT.float32)
                T1 = mpool.tile([18, 2048], DT.float32)
                T2 = mpool.tile([18, 2048], DT.float32)
                T3 = mpool.tile([18, 2048], DT.float32)
                cst = mpool.tile([18, 4], DT.float32)

                pidx = mpool.tile([32, 4], DT.float32)
                _iotas.append(nc.gpsimd.iota(pidx[:, 0:1], pattern=[[0, 1]],
                               channel_multiplier=1,
                               allow_small_or_imprecise_dtypes=True))
                P18 = pidx[0:18, 0:1]
                hh, kk, kh3, km3 = (cst[:, i : i + 1] for i in range(4))
                nc.vector.tensor_scalar(out=hh, in0=P18, scalar1=8.5, scalar2=None, op0=ALU.is_gt)
                nc.vector.tensor_scalar(out=kk, in0=hh, scalar1=-9.0, scalar2=None, op0=ALU.mult)
                nc.vector.tensor_add(kk, kk, P18)
                t_a = pidx[0:18, 1:2]
                nc.vector.tensor_scalar(out=kh3, in0=kk, scalar1=2.5, scalar2=None, op0=ALU.is_gt)
                nc.vector.tensor_scalar(out=t_a, in0=kk, scalar1=5.5, scalar2=None, op0=ALU.is_gt)
                nc.vector.tensor_add(kh3, kh3, t_a)
                nc.vector.tensor_scalar(out=km3, in0=kh3, scalar1=-3.0, scalar2=None, op0=ALU.mult)
                nc.vector.tensor_add(km3, km3, kk)
                cstv = mpool.tile([18, 4], DT.float32, tag="cstv")
                nc.vector.tensor_scalar(out=cstv[:, 0:1], in0=hh, scalar1=64.0, scalar2=511.0,
                                        op0=ALU.mult, op1=ALU.add)
                nc.vector.tensor_add(cstv[:, 0:1], cstv[:, 0:1], kh3)
                nc.vector.tensor_scalar(out=cstv[:, 1:2], in0=km3, scalar1=511.0, scalar2=None, op0=ALU.add)
                nc.vector.tensor_scalar(out=cstv[:, 2:3], in0=hh, scalar1=-64.0, scalar2=6.0 - 512.0,
                                        op0=ALU.mult, op1=ALU.add)

                MC = 2048
                for cc in range(NPIX // MC):
                    cs = slice(cc * MC, (cc + 1) * MC)
                    for role, dstt in ((0, OY), (1, OX), (2, OM)):
                        for h in range(2):
                            nc.sync.dma_start(
                                out=dstt[h * 9 : h * 9 + 9, :],
                                in_=om[h * 27 + role * 9 : h * 27 + role * 9 + 9, cs],
                            )
                    # py = OY + rowbase ; fy = mod(py,1); y0f = py - fy
                    _iotas.append(nc.gpsimd.iota(IOT[:], pattern=[[1, MC // W], [0, W]],
                                   base=cc * (MC // W),
                                   channel_multiplier=0,
                                   allow_small_or_imprecise_dtypes=True))
                    nc.vector.tensor_add(T0[:], OY[:], IOT[:])
                    nc.vector.tensor_scalar(out=T0[:], in0=T0[:], scalar1=cstv[:, 0:1],
                                            scalar2=None, op0=ALU.add)
                    nc.vector.tensor_scalar(out=T2[:], in0=T0[:], scalar1=8388608.0, scalar2=-8388608.0,
                                            op0=ALU.add, op1=ALU.add)
                    nc.vector.tensor_tensor(out=OY[:], in0=T2[:], in1=T0[:], op=ALU.is_gt)
                    nc.vector.tensor_sub(T2[:], T2[:], OY[:])
                    nc.vector.tensor_sub(OY[:], T0[:], T2[:])
                    nc.vector.tensor_copy(out=T0[:], in_=T2[:])
                    _iotas.append(nc.gpsimd.iota(IOT[:], pattern=[[0, MC // W], [1, W]],
                                   channel_multiplier=0,
                                   allow_small_or_imprecise_dtypes=True))
                    nc.vector.tensor_add(T1[:], OX[:], IOT[:])
                    nc.vector.tensor_scalar(out=T1[:], in0=T1[:], scalar1=cstv[:, 1:2],
                                            scalar2=None, op0=ALU.add)
                    nc.vector.tensor_scalar(out=T2[:], in0=T1[:], scalar1=8388608.0, scalar2=-8388608.0,
                                            op0=ALU.add, op1=ALU.add)
                    nc.vector.tensor_tensor(out=OX[:], in0=T2[:], in1=T1[:], op=ALU.is_gt)
                    nc.vector.tensor_sub(T2[:], T2[:], OX[:])
                    nc.vector.tensor_sub(OX[:], T1[:], T2[:])
                    nc.vector.tensor_copy(out=T1[:], in_=T2[:])

                    nc.vector.tensor_scalar(out=T0[:], in0=T0[:], scalar1=cstv[:, 2:3],
                                            scalar2=None, op0=ALU.add)
                    nc.vector.tensor_scalar(out=T0[:], in0=T0[:], scalar1=0.0, scalar2=75.0,
                                            op0=ALU.max, op1=ALU.min)
                    nc.vector.tensor_scalar_mul(out=T0[:], in0=T0[:], scalar1=0.5)
                    nc.vector.tensor_scalar(out=T3[:], in0=T0[:], scalar1=8388608.0, scalar2=-8388608.0,
                                            op0=ALU.add, op1=ALU.add)
                    nc.vector.tensor_tensor(out=T2[:], in0=T3[:], in1=T0[:], op=ALU.is_gt)
                    nc.vector.tensor_sub(T3[:], T3[:], T2[:])
                    nc.vector.tensor_sub(T2[:], T0[:], T3[:])
                    nc.vector.tensor_copy(out=T0[:], in_=T3[:])
                    nc.vector.tensor_scalar(out=T1[:], in0=T1[:], scalar1=2.0 - 512.0,
                                            scalar2=None, op0=ALU.add)
                    nc.vector.tensor_scalar(out=T1[:], in0=T1[:], scalar1=0.0, scalar2=130.0,
                                            op0=ALU.max, op1=ALU.min)
                    nc.vector.tensor_scalar_mul(out=T1[:], in0=T1[:], scalar1=0.5)
                    nc.vector.tensor_scalar(out=IOT[:], in0=T1[:], scalar1=8388608.0, scalar2=-8388608.0,
                                            op0=ALU.add, op1=ALU.add)
                    nc.vector.tensor_tensor(out=T3[:], in0=IOT[:], in1=T1[:], op=ALU.is_gt)
                    nc.vector.tensor_sub(IOT[:], IOT[:], T3[:])
                    nc.vector.tensor_sub(T3[:], T1[:], IOT[:])
                    nc.vector.tensor_copy(out=T1[:], in_=IOT[:])

                    nc.vector.tensor_scalar_mul(out=T2[:], in0=T2[:], scalar1=float(4 * RY_N * RX_N))
                    nc.vector.tensor_scalar_mul(out=T3[:], in0=T3[:], scalar1=float(2 * RY_N * RX_N))
                    nc.vector.tensor_add(T2[:], T2[:], T3[:])
                    nc.vector.tensor_scalar_mul(out=T0[:], in0=T0[:], scalar1=float(RX_N))
                    nc.vector.tensor_add(T2[:], T2[:], T0[:])
                    nc.vector.tensor_add(T2[:], T2[:], T1[:])
                    nc.vector.tensor_copy(out=idx16[:, cs], in_=T2[:])

                    nc.scalar.activation(out=OM[:], in_=OM[:], func=AF.Sigmoid)
                    nc.vector.tensor_scalar_mul(out=OM[:], in0=OM[:], scalar1=2.0)
                    nc.vector.tensor_scalar(out=T0[:], in0=OY[:], scalar1=-1.0, scalar2=1.0,
                                            op0=ALU.mult, op1=ALU.add)
                    nc.vector.tensor_scalar(out=T1[:], in0=OX[:], scalar1=-1.0, scalar2=1.0,
                                            op0=ALU.mult, op1=ALU.add)
                    for qi, (ya, xa) in enumerate(((T0, T1), (T0, OX), (OY, T1), (OY, OX))):
                        nc.vector.tensor_mul(T2[:], ya[:], xa[:])
                        nc.vector.tensor_mul(T2[:], T2[:], OM[:])
                        nc.vector.tensor_copy(out=wq[32 * qi : 32 * qi + 18, cs], in_=T2[:])

                # idx16 -> wrapped per-(k,h) layout via DRAM bounce
                idx_dram = nc.dram_tensor("idx_scratch", [18, NPIX], DT.int16)
                nc.sync.dma_start(out=idx_dram[:], in_=idx16[:])
                for k in range(KF):
                    for h in range(2):
                        srcv = idx_dram[h * 9 + k : h * 9 + k + 1, :].rearrange(
                            "p (s l) -> (p l) s", l=16)
                        for g in range(4):
                            p0 = h * 64 + g * 16
                            nc.sync.dma_start(
                                out=idxt[p0 : p0 + 16, k * S16 : (k + 1) * S16],
                                in_=srcv,
                            )

        # selector lhsT: sel[32*qi + j, qi*128 + (j//9)*64 + o] = 1 for j<18
        with tc.tile_pool(name="selb", bufs=1) as selb:
            rP = selb.tile([128, 1], DT.float32)
            cC = selb.tile([128, 512], DT.float32)
            t1 = selb.tile([128, 512], DT.float32)
            t2 = selb.tile([128, 512], DT.float32)
            _iotas.append(nc.gpsimd.iota(rP[:], pattern=[[0, 1]], channel_multiplier=1,
                           allow_small_or_imprecise_dtypes=True))
            _iotas.append(nc.gpsimd.iota(cC[:], pattern=[[1, 512]], channel_multiplier=0,
                           allow_small_or_imprecise_dtypes=True))
            # j = r mod 32 ; qi_r = (r - j)/32
            j32 = selb.tile([128, 1], DT.float32)
            qir = selb.tile([128, 1], DT.float32)
            jt = selb.tile([128, 1], DT.float32)
            nc.vector.tensor_scalar(out=qir[:], in0=rP[:], scalar1=31.5, scalar2=None, op0=ALU.is_gt)
            nc.vector.tensor_scalar(out=jt[:], in0=rP[:], scalar1=63.5, scalar2=None, op0=ALU.is_gt)
            nc.vector.tensor_add(qir[:], qir[:], jt[:])
            nc.vector.tensor_scalar(out=jt[:], in0=rP[:], scalar1=95.5, scalar2=None, op0=ALU.is_gt)
            nc.vector.tensor_add(qir[:], qir[:], jt[:])
            nc.vector.tensor_scalar(out=j32[:], in0=qir[:], scalar1=-32.0, scalar2=None, op0=ALU.mult)
            nc.vector.tensor_add(j32[:], j32[:], rP[:])
            # cond1: floor(c/128) == qi_r  -> |c/128 - qi_r - frac| via mod
            t3 = selb.tile([128, 512], DT.float32)
            nc.vector.tensor_scalar(out=t2[:], in0=cC[:], scalar1=127.5, scalar2=None, op0=ALU.is_gt)
            nc.vector.tensor_scalar(out=t3[:], in0=cC[:], scalar1=255.5, scalar2=None, op0=ALU.is_gt)
            nc.vector.tensor_add(t2[:], t2[:], t3[:])
            nc.vector.tensor_scalar(out=t3[:], in0=cC[:], scalar1=383.5, scalar2=None, op0=ALU.is_gt)
            nc.vector.tensor_add(t2[:], t2[:], t3[:])   # floor(c/128)
            nc.vector.tensor_scalar(out=t1[:], in0=t2[:], scalar1=-128.0, scalar2=None, op0=ALU.mult)
            nc.vector.tensor_add(t1[:], t1[:], cC[:])   # c mod 128
            nc.vector.tensor_scalar(out=t2[:], in0=t2[:], scalar1=qir[:], scalar2=None,
                                    op0=ALU.is_equal)
            # cond2: floor((c mod 128)/64) == floor(j/9)  (j<18 -> floor(j/9) in {0,1})
            nc.vector.tensor_scalar(out=t1[:], in0=t1[:], scalar1=63.5, scalar2=None,
                                    op0=ALU.is_gt)             # h(c)
            hj = selb.tile([128, 1], DT.float32)
            nc.vector.tensor_scalar(out=hj[:], in0=j32[:], scalar1=8.5, scalar2=None,
                                    op0=ALU.is_gt)             # j>=9
            nc.vector.tensor_scalar(out=t1[:], in0=t1[:], scalar1=hj[:], scalar2=None,
                                    op0=ALU.is_equal)
            nc.vector.tensor_mul(t2[:], t2[:], t1[:])
            # cond3: j < 18
            j18 = selb.tile([128, 1], DT.float32)
            nc.vector.tensor_scalar(out=j18[:], in0=j32[:], scalar1=17.5, scalar2=None,
                                    op0=ALU.is_lt)
            nc.vector.tensor_scalar(out=t2[:], in0=t2[:], scalar1=j18[:], scalar2=None,
                                    op0=ALU.mult)
            # per-k selectivity: jk = j32 - 9*hj ; sel_k = t2 * (jk == k)
            jkk = selb.tile([128, 1], DT.float32)
            nc.vector.tensor_scalar(out=jkk[:], in0=hj[:], scalar1=-9.0, scalar2=None,
                                    op0=ALU.mult)
            nc.vector.tensor_add(jkk[:], jkk[:], j32[:])
            tk = selb.tile([128, 1], DT.float32)
            for k in range(KF):
                nc.vector.tensor_scalar(out=tk[:], in0=jkk[:], scalar1=float(k),
                                        scalar2=None, op0=ALU.is_equal)
                nc.vector.tensor_scalar(out=sel[:, k * 512 : (k + 1) * 512],
                                        in0=t2[:], scalar1=tk[:, 0:1],
                                        scalar2=None, op0=ALU.mult)

        # wdup + b_dc
        nc.vector.memset(wdup[:], 0.0)
        wdc_v = wdc_ext[:].rearrange("o c kh kw -> c o (kh kw)")
        for k in range(KF):
            for h in range(2):
                nc.gpsimd.dma_start(
                    out=wdup[h * 64 : h * 64 + 64,
                             k * 128 + h * 64 : k * 128 + h * 64 + 64],
                    in_=wdc_v[:, :, k : k + 1].rearrange("c a d -> c (a d)"),
                )
        for h in range(2):
            nc.sync.dma_start(
                out=bdc_t[h * 64 : h * 64 + 64, 0:1],
                in_=bdc_ext[:].rearrange("(o one) -> o one", one=1),
            )

        # ======== phase 4+5 in one pool scope
        with (
            tc.tile_pool(name="qt", bufs=1) as qtp,
            tc.tile_pool(name="g", bufs=2) as gpool,
            tc.tile_pool(name="h", bufs=2) as hpool,
            tc.tile_pool(name="o", bufs=2) as opool,
            tc.tile_pool(name="mp", bufs=4, space="PSUM") as mpsum,
            tc.tile_pool(name="op", bufs=1, space="PSUM") as opsum,
        ):
            _lib = nc.gpsimd.load_library(library_config.ap_gather)
            for _io in _iotas:
                tile.add_dep_helper(_lib.ins, _io.ins, reason="lib load after iotas")
            qtab = qtp.tile([128, NBLK * 4], DT.bfloat16)
            nc.vector.memset(qtab[:], 0.0)
            q4 = qtab[:].rearrange("p (blk q) -> p blk q", q=4)
            for a in range(2):
                for b in range(2):
                    blk0 = (a * 2 + b) * (RY_N * RX_N)
                    for qy in range(2):
                        for qx in range(2):
                            ry_cnt = min((75 - a - qy) // 2 + 1, RY_N)
                            rx0 = 1 if (b + qx) == 0 else 0
                            rx1 = min(RX_N - 1, (130 - b - qx) // 2)
                            rx_cnt = rx1 - rx0 + 1
                            c0 = 2 * rx0 + b + qx - 1
                            src = x3()[:, a + qy : a + qy + 2 * (ry_cnt - 1) + 1 : 2,
                                       c0 : c0 + 2 * (rx_cnt - 1) + 1 : 2]
                            dst3 = q4[:, blk0 + rx0 : blk0 + rx0 + (ry_cnt - 1) * RX_N + rx_cnt,
                                      qy * 2 + qx : qy * 2 + qx + 1]
                            dst = bass.AP(
                                dst3.tensor, dst3.offset,
                                [dst3.ap[0], [RX_N * 4, ry_cnt], [4, rx_cnt]],
                            )
                            nc.vector.tensor_copy(out=dst, in_=src)

            qtab_f32 = qtab[:].bitcast(DT.float32)
            outv = out_ext[:].rearrange("o h w -> o (h w)")

            for cb in range(NPIX // GCHUNK):
                po = opsum.tile([128, GCHUNK], DT.float32, tag="po")
                for k in range(KF):
                    g = gpool.tile([128, GCHUNK * 2], DT.float32, tag="g")
                    idx_sl = idxt[:, k * S16 + cb * (GCHUNK // 16):
                                  k * S16 + (cb + 1) * (GCHUNK // 16)]
                    _ga = nc.gpsimd.ap_gather(
                        g[:], qtab_f32, idx_sl,
                        channels=128, num_elems=NBLK, d=2, num_idxs=GCHUNK,
                    )
                    tile.add_dep_helper(_ga.ins, _lib.ins, reason="gather after lib load")
                    gb = g[:].bitcast(DT.bfloat16).rearrange(
                        "p (n q) -> p n q", q=4
                    )
                    for sub in range(GCHUNK // SUB):
                        col0 = cb * GCHUNK + sub * SUB
                        hts = []
                        for qi in range(4):
                            mq = mpsum.tile([128, SUB], DT.float32, tag="mq")
                            nc.tensor.matmul(
                                out=mq[:],
                                lhsT=sel[:, k * 512 + qi * 128 : k * 512 + (qi + 1) * 128],
                                rhs=wq[:, col0 : col0 + SUB],
                                start=True, stop=True,
                            )
                            ht = hpool.tile([128, SUB], DT.bfloat16, tag=f"ht{qi}")
                            gq = gb[:, sub * SUB : (sub + 1) * SUB,
                                    qi : qi + 1].rearrange("p n one -> p (n one)")
                            nc.vector.tensor_mul(ht[:], mq[:], gq)
                            hts.append(ht)
                        for qi in range(4):
                            nc.tensor.matmul(
                                out=po[:, sub * SUB : (sub + 1) * SUB],
                                lhsT=wdup[:, k * 128 : (k + 1) * 128],
                                rhs=hts[qi][:],
                                start=(k == 0 and qi == 0),
                                stop=(k == KF - 1 and qi == 3),
                            )
                ot = opool.tile([128, GCHUNK], DT.float32, tag="ot")
                nc.vector.tensor_scalar(
                    out=ot[:], in0=po[:], scalar1=bdc_t[:, 0:1],
                    scalar2=None, op0=ALU.add,
                )
                for h in range(2):
                    nc.sync.dma_start(
                        out=outv[:, h * NPIX + cb * GCHUNK:
                                 h * NPIX + (cb + 1) * GCHUNK],
                        in_=ot[h * 64 : h * 64 + 64, :],
                    )


def _build_nc():
    _install_compat()
    nc = bass.Bass()
    x_ext = nc.declare_dram_parameter("x", [C, H, W], DT.float32, isOutput=False)
    wom_ext = nc.declare_dram_parameter("w_om", [3 * KF, C, K, K], DT.float32, isOutput=False)
    bom_ext = nc.declare_dram_parameter("b_om", [3 * KF], DT.float32, isOutput=False)
    wdc_ext = nc.declare_dram_parameter("w_dc", [O, C, K, K], DT.float32, isOutput=False)
    bdc_ext = nc.declare_dram_parameter("b_dc", [O], DT.float32, isOutput=False)
    out_ext = nc.declare_dram_parameter("out", [O, H, W], DT.float32, isOutput=True)
    with tile.TileContext(nc) as tc:
        _emit(nc, tc, x_ext, wom_ext, bom_ext, wdc_ext, bdc_ext, out_ext)
    lower_extended_insts(nc)
    return nc


_NC_CACHE = None


def kernel(**inputs):
    global _NC_CACHE
    x = np.ascontiguousarray(inputs["x"], dtype=np.float32)
    w_om = np.ascontiguousarray(inputs["w_om"], dtype=np.float32)
    b_om = np.ascontiguousarray(inputs["b_om"], dtype=np.float32)
    w_dc = np.ascontiguousarray(inputs["w_dc"], dtype=np.float32)
    b_dc = np.ascontiguousarray(inputs["b_dc"], dtype=np.float32)

    if _NC_CACHE is None:
        _NC_CACHE = _build_nc()
    nc = _NC_CACHE

    in_maps = [
        {"x": x[i], "w_om": w_om, "b_om": b_om, "w_dc": w_dc, "b_dc": b_dc}
        for i in range(NCORES)
    ]
    res = run_bass_kernel_spmd(nc, in_maps, core_ids=list(range(NCORES)))
    return np.stack(
        [np.asarray(res.results[i]["out"]) for i in range(NCORES)]
    ).astype(np.float32)



# revision 5
# speedup vs baseline: 1.3964x; 1.3964x over previous
"""Trainium2 Bass kernel for nn_AdaFeatBlock (modulated deformable-conv block).

Sharding: data-parallel over batch — 8 samples -> 8 NeuronCores, all weights
replicated; each core computes its sample end-to-end, host stacks outputs.

Per-core pipeline (one sample, x [64,128,128]):
  1. x -> bf16 "half-split" padded layout: partition h*64+c = channel c of
     image-half h; free = 76 stored rows (h*64-6 .. h*64+69) x 130 cols
     (-1..128), zero-padded borders.
  2. offset/mask 3x3 conv = 9 shifted matmuls, block-diagonal [128, 54]
     lhsT (both halves at once), PSUM-accumulated. Output row order per
     half: [off_y k0..8 | off_x k0..8 | mask k0..8].
  3. Coordinate math on [18, 8192] tiles (partition = (half, k)): bilinear
     corner weights (mask-modulated, zero outside the image via the
     zero-padded gather table) and 2x2-quad table indices.
  4. Quad gather table Q[128, 10032, 4] bf16: 2x2 pixel blocks at all 4
     row/col parities. ap_gather (d=2 f32 view = 8B quad) fetches a
     point's 4 corners for 16 channels/core; 8 Q7 cores cover
     128 partitions = 2 halves x 64 channels.
  5. Per (k, 512-px block): corner-weight rows broadcast to 128 partitions
     via a tiny selector matmul into PSUM; fused PSUM-read multiply into
     gathered corners; 3 adds -> modulated bilinear sample "val".
  6. Deformable einsum: per k a [128->128, 512] matmul with block-diagonal
     channel-duplicated w_dc, PSUM-accumulated over all 9 k. + b_dc -> out.
"""

import numpy as np

import concourse.bass as bass
import concourse.tile as tile
from concourse import mybir
from concourse.bass_utils import run_bass_kernel_spmd
from concourse import library_config
from concourse.library_overlay import lower_extended_insts
from concourse.vector_clock import ScopedClock

AF = mybir.ActivationFunctionType
ALU = mybir.AluOpType
DT = mybir.dt

B, C, H, W = 8, 64, 128, 128
O = 64
K = 3
KF = 9
NCORES = 8
HALF = H // 2
NPIX = H * W // 2              # 8192 pixels per half
ROWS_ST = 76                   # stored rows per half
PITCH = 130                    # stored cols (-1..128)
RY_N, RX_N = 38, 66
NBLK = 4 * RY_N * RX_N         # 10032
GCHUNK = 2048
SUB = 512
S16 = NPIX // 16               # idx ints per partition per k


def _install_compat():
    """This walrus build accepts at most ONE sync-wait per instruction."""
    if getattr(tile.TileContext, "_adafeat_patched", False):
        return
    _orig_lower = tile.TileContext._lower_ordered_insts

    def _split_waits(nc, ordered):
        for insts in ordered.values():
            new_insts = []
            for inst in insts:
                si = inst.sync_info
                if si is not None and si.on_wait and len(si.on_wait) > 1:
                    waits = list(si.on_wait)
                    for w in waits[:-1]:
                        nop = mybir.InstNoOp(name=f"I-{nc.next_id()}", ins=[], outs=[])
                        nop.engine = inst.engine
                        nop.sync_info = mybir.SyncInfo(on_wait=[w], on_update=[])
                        new_insts.append(nop)
                    inst.sync_info = mybir.SyncInfo(
                        on_wait=[waits[-1]], on_update=list(si.on_update)
                    )
                new_insts.append(inst)
            insts[:] = new_insts

    def _lower_split(self, ordered):
        _split_waits(self.nc, ordered)
        return _orig_lower(self, ordered)

    def _drain_split(self, tick_clock, wait_clock):
        carrier = self.nc.sync.nop(nofuse=True)
        wait_clock.add_sem_waits(
            carrier.ins, ScopedClock({None: tick_clock.global_clock})
        )
        si = carrier.ins.sync_info
        if si is not None and si.on_wait and len(si.on_wait) > 1:
            waits = list(si.on_wait)
            carrier.ins.sync_info = mybir.SyncInfo(
                on_wait=waits[:1], on_update=list(si.on_update)
            )
            for w in waits[1:]:
                extra = self.nc.sync.nop(nofuse=True)
                extra.ins.sync_info = mybir.SyncInfo(on_wait=[w], on_update=[])
        self.nc.sync.drain()
        self.nc.all_engine_barrier()
        popped = self.nc._tile_sem_poison_stack.pop()
        assert popped is self._sem_poison
        self.nc.clear_and_free_semaphores(list(self.sems.allocated().values()))
        self.nc.all_engine_barrier()

    tile.TileContext._lower_ordered_insts = _lower_split
    tile.TileContext._drain_and_barrier = _drain_split
    tile.TileContext._adafeat_patched = True


def _emit(nc, tc, x_ext, wom_ext, bom_ext, wdc_ext, bdc_ext, out_ext):
    _iotas = []

    with tc.tile_pool(name="persist", bufs=1) as persist:
        x_sb = persist.tile([128, ROWS_ST * PITCH], DT.bfloat16)
        wq = persist.tile([128, NPIX], DT.bfloat16)
        idxt = persist.tile([128, KF * S16], DT.int16)
        wdup = persist.tile([128, KF * 128], DT.bfloat16)
        sel = persist.tile([128, KF * 4 * 128], DT.bfloat16)
        bdc_t = persist.tile([128, 1], DT.float32)

        x3 = lambda: x_sb[:].rearrange("p (r c) -> p r c", c=PITCH)

        # ======== phase 1: load x (f32 -> bf16), half-split, zero-padded
        nc.vector.memset(x_sb[:], 0.0)
        nc.vector.memset(wq[:], 0.0)
        xv = x_ext[:]
        for h in range(2):
            r0 = max(0, h * HALF - 6)
            r1 = min(H - 1, h * HALF + 69)
            nrow = r1 - r0 + 1
            rloc = r0 - (h * HALF - 6)
            dst = x3()[h * 64 : h * 64 + 64, rloc : rloc + nrow, 1 : 1 + W]
            nc.gpsimd.dma_start(out=dst, in_=xv[:, r0 : r1 + 1, :])

        # ======== phase 2: offset/mask conv
        with (
            tc.tile_pool(name="convw", bufs=1) as convw,
            tc.tile_pool(name="omp", bufs=1) as omp,
            tc.tile_pool(name="convp", bufs=2, space="PSUM") as convp,
        ):
            # w_om views: y/x roles from rows 0..17 (o=2k+r), m role rows 18..26
            wom_yx = wom_ext[:][0:18].rearrange(
                "(o2 r) c kh kw -> c o2 r (kh kw)", r=2
            )
            wom_m = wom_ext[:][18:27].rearrange("o c kh kw -> c o (kh kw)")
            lhs_om = []
            for dy in range(3):
                for dx in range(3):
                    dd = dy * K + dx
                    t = convw.tile([128, 54], DT.bfloat16, tag=f"lom{dd}")
                    nc.vector.memset(t[:], 0.0)
                    for h in range(2):
                        ps = slice(h * 64, h * 64 + 64)
                        nc.gpsimd.dma_start(
                            out=t[ps, h * 27 + 0 : h * 27 + 9],
                            in_=wom_yx[:, 0:9, 0:1, dd : dd + 1].rearrange(
                                "c a b d -> c (a b d)"),
                        )
                        nc.gpsimd.dma_start(
                            out=t[ps, h * 27 + 9 : h * 27 + 18],
                            in_=wom_yx[:, 0:9, 1:2, dd : dd + 1].rearrange(
                                "c a b d -> c (a b d)"),
                        )
                        nc.gpsimd.dma_start(
                            out=t[ps, h * 27 + 18 : h * 27 + 27],
                            in_=wom_m[:, :, dd : dd + 1].rearrange(
                                "c a d -> c (a d)"),
                        )
                    lhs_om.append(t)

            bom_t = convw.tile([54, 1], DT.float32)
            bom_yx = bom_ext[:][0:18].rearrange("(o2 r) -> o2 r", r=2)
            for h in range(2):
                nc.sync.dma_start(
                    out=bom_t[h * 27 + 0 : h * 27 + 9, 0:1], in_=bom_yx[0:9, 0:1]
                )
                nc.sync.dma_start(
                    out=bom_t[h * 27 + 9 : h * 27 + 18, 0:1], in_=bom_yx[0:9, 1:2]
                )
                nc.sync.dma_start(
                    out=bom_t[h * 27 + 18 : h * 27 + 27, 0:1],
                    in_=bom_ext[:][18:27].rearrange("(o one) -> o one", one=1),
                )

            om = omp.tile([54, NPIX], DT.float32)
            rows_per_sub = SUB // W  # 4
            for cb in range(NPIX // SUB):
                pt = convp.tile([54, SUB], DT.float32, tag="cpt")
                r0 = cb * rows_per_sub
                for i, (dy, dx) in enumerate(
                    (dy, dx) for dy in range(3) for dx in range(3)
                ):
                    rhs = x3()[:, 6 + r0 + dy - 1 : 6 + r0 + dy - 1 + rows_per_sub,
                               dx : dx + W]
                    nc.tensor.matmul(
                        out=pt[:], lhsT=lhs_om[i][:], rhs=rhs,
                        start=(i == 0), stop=(i == 8),
                    )
                nc.vector.tensor_scalar(
                    out=om[:, cb * SUB : (cb + 1) * SUB], in0=pt[:],
                    scalar1=bom_t[:, 0:1], scalar2=None, op0=ALU.add,
                )

            # ======== phase 3: coordinate math, single pass on [72, 2048]
            # partition P = cc*32 + h*9 + k  (cc = 2048-px chunk of the half; 32-aligned blocks)
            with tc.tile_pool(name="math", bufs=1) as mpool:
                idx16 = mpool.tile([18, NPIX], DT.int16)
                OY = mpool.tile([128, 2048], DT.float32)
                OX = mpool.tile([128, 2048], DT.float32)
                OM = mpool.tile([128, 2048], DT.float32)
                IOT = mpool.tile([128, 2048], DT.float32)
                T0 = mpool.tile([128, 2048], DT.float32)
                T1 = mpool.tile([128, 2048], DT.float32)
                T2 = mpool.tile([128, 2048], DT.float32)
                T3 = mpool.tile([128, 2048], DT.float32)
                cst = mpool.tile([128, 6], DT.float32)

                pidx = mpool.tile([128, 4], DT.float32)
                _iotas.append(nc.gpsimd.iota(pidx[:, 0:1], pattern=[[0, 1]],
                               channel_multiplier=1,
                               allow_small_or_imprecise_dtypes=True))
                P128 = pidx[:, 0:1]
                hh, kk, kh3, km3, ccv, hkv = (cst[:, i : i + 1] for i in range(6))
                t_a = pidx[:, 1:2]
                # cc = P // 18
                nc.vector.tensor_scalar(out=ccv, in0=P128, scalar1=31.5, scalar2=None, op0=ALU.is_gt)
                nc.vector.tensor_scalar(out=t_a, in0=P128, scalar1=63.5, scalar2=None, op0=ALU.is_gt)
                nc.vector.tensor_add(ccv, ccv, t_a)
                nc.vector.tensor_scalar(out=t_a, in0=P128, scalar1=95.5, scalar2=None, op0=ALU.is_gt)
                nc.vector.tensor_add(ccv, ccv, t_a)
                # hk = P - 18*cc ; h = hk > 8.5 ; k = hk - 9*h
                nc.vector.tensor_scalar(out=hkv, in0=ccv, scalar1=-32.0, scalar2=None, op0=ALU.mult)
                nc.vector.tensor_add(hkv, hkv, P128)
                nc.vector.tensor_scalar(out=hh, in0=hkv, scalar1=8.5, scalar2=None, op0=ALU.is_gt)
                nc.vector.tensor_scalar(out=kk, in0=hh, scalar1=-9.0, scalar2=None, op0=ALU.mult)
                nc.vector.tensor_add(kk, kk, hkv)
                nc.vector.tensor_scalar(out=kh3, in0=kk, scalar1=2.5, scalar2=None, op0=ALU.is_gt)
                nc.vector.tensor_scalar(out=t_a, in0=kk, scalar1=5.5, scalar2=None, op0=ALU.is_gt)
                nc.vector.tensor_add(kh3, kh3, t_a)
                nc.vector.tensor_scalar(out=km3, in0=kh3, scalar1=-3.0, scalar2=None, op0=ALU.mult)
                nc.vector.tensor_add(km3, km3, kk)
                cstv = mpool.tile([128, 4], DT.float32, tag="cstv")
                # cstv0 = 64*h + 511 + kh3 + 16*cc
                nc.vector.tensor_scalar(out=cstv[:, 0:1], in0=hh, scalar1=64.0, scalar2=511.0,
                                        op0=ALU.mult, op1=ALU.add)
                nc.vector.tensor_add(cstv[:, 0:1], cstv[:, 0:1], kh3)
                nc.vector.tensor_scalar(out=t_a, in0=ccv, scalar1=16.0, scalar2=None, op0=ALU.mult)
                nc.vector.tensor_add(cstv[:, 0:1], cstv[:, 0:1], t_a)
                nc.vector.tensor_scalar(out=cstv[:, 1:2], in0=km3, scalar1=511.0, scalar2=None, op0=ALU.add)
                nc.vector.tensor_scalar(out=cstv[:, 2:3], in0=hh, scalar1=-64.0, scalar2=6.0 - 512.0,
                                        op0=ALU.mult, op1=ALU.add)

                MC = 2048
                for cc in range(NPIX // MC):
                    cs = slice(cc * MC, (cc + 1) * MC)
                    for role, dstt in ((0, OY), (1, OX), (2, OM)):
                        for h in range(2):
                            nc.sync.dma_start(
                                out=dstt[cc * 32 + h * 9 : cc * 32 + h * 9 + 9, :],
                                in_=om[h * 27 + role * 9 : h * 27 + role * 9 + 9, cs],
                            )
                # py = OY + rowbase ; fy = mod(py,1); y0f = py - fy
                _iotas.append(nc.gpsimd.iota(IOT[:], pattern=[[1, MC // W], [0, W]],
                               channel_multiplier=0,
                               allow_small_or_imprecise_dtypes=True))
                nc.vector.tensor_add(T0[:], OY[:], IOT[:])
                nc.vector.tensor_scalar(out=T0[:], in0=T0[:], scalar1=cstv[:, 0:1],
                                        scalar2=None, op0=ALU.add)
                nc.vector.tensor_scalar(out=T2[:], in0=T0[:], scalar1=8388608.0, scalar2=-8388608.0,
                                        op0=ALU.add, op1=ALU.add)
                nc.vector.tensor_tensor(out=OY[:], in0=T2[:], in1=T0[:], op=ALU.is_gt)
                nc.vector.tensor_sub(T2[:], T2[:], OY[:])
                nc.vector.tensor_sub(OY[:], T0[:], T2[:])
                nc.vector.tensor_copy(out=T0[:], in_=T2[:])
                _iotas.append(nc.gpsimd.iota(IOT[:], pattern=[[0, MC // W], [1, W]],
                               channel_multiplier=0,
                               allow_small_or_imprecise_dtypes=True))
                nc.vector.tensor_add(T1[:], OX[:], IOT[:])
                nc.vector.tensor_scalar(out=T1[:], in0=T1[:], scalar1=cstv[:, 1:2],
                                        scalar2=None, op0=ALU.add)
                nc.vector.tensor_scalar(out=T2[:], in0=T1[:], scalar1=8388608.0, scalar2=-8388608.0,
                                        op0=ALU.add, op1=ALU.add)
                nc.vector.tensor_tensor(out=OX[:], in0=T2[:], in1=T1[:], op=ALU.is_gt)
                nc.vector.tensor_sub(T2[:], T2[:], OX[:])
                nc.vector.tensor_sub(OX[:], T1[:], T2[:])
                nc.vector.tensor_copy(out=T1[:], in_=T2[:])

                nc.vector.tensor_scalar(out=T0[:], in0=T0[:], scalar1=cstv[:, 2:3],
                                        scalar2=None, op0=ALU.add)
                nc.vector.tensor_scalar(out=T0[:], in0=T0[:], scalar1=0.0, scalar2=75.0,
                                        op0=ALU.max, op1=ALU.min)
                nc.vector.tensor_scalar_mul(out=T0[:], in0=T0[:], scalar1=0.5)
                nc.vector.tensor_scalar(out=T3[:], in0=T0[:], scalar1=8388608.0, scalar2=-8388608.0,
                                        op0=ALU.add, op1=ALU.add)
                nc.vector.tensor_tensor(out=T2[:], in0=T3[:], in1=T0[:], op=ALU.is_gt)
                nc.vector.tensor_sub(T3[:], T3[:], T2[:])
                nc.vector.tensor_sub(T2[:], T0[:], T3[:])
                nc.vector.tensor_copy(out=T0[:], in_=T3[:])
                nc.vector.tensor_scalar(out=T1[:], in0=T1[:], scalar1=2.0 - 512.0,
                                        scalar2=None, op0=ALU.add)
                nc.vector.tensor_scalar(out=T1[:], in0=T1[:], scalar1=0.0, scalar2=130.0,
                                        op0=ALU.max, op1=ALU.min)
                nc.vector.tensor_scalar_mul(out=T1[:], in0=T1[:], scalar1=0.5)
                nc.vector.tensor_scalar(out=IOT[:], in0=T1[:], scalar1=8388608.0, scalar2=-8388608.0,
                                        op0=ALU.add, op1=ALU.add)
                nc.vector.tensor_tensor(out=T3[:], in0=IOT[:], in1=T1[:], op=ALU.is_gt)
                nc.vector.tensor_sub(IOT[:], IOT[:], T3[:])
                nc.vector.tensor_sub(T3[:], T1[:], IOT[:])
                nc.vector.tensor_copy(out=T1[:], in_=IOT[:])

                nc.vector.tensor_scalar_mul(out=T2[:], in0=T2[:], scalar1=float(4 * RY_N * RX_N))
                nc.vector.tensor_scalar_mul(out=T3[:], in0=T3[:], scalar1=float(2 * RY_N * RX_N))
                nc.vector.tensor_add(T2[:], T2[:], T3[:])
                nc.vector.tensor_scalar_mul(out=T0[:], in0=T0[:], scalar1=float(RX_N))
                nc.vector.tensor_add(T2[:], T2[:], T0[:])
                nc.vector.tensor_add(T2[:], T2[:], T1[:])
                # write idx16 in WRAPPED col order: per 2048-chunk,
                # idx16[r, cc*2048 + p*128 + c] = idx of pixel cc*2048 + c*16 + p
                for cc in range(NPIX // MC):
                    nc.vector.tensor_copy(
                        out=idx16[:, cc * MC : (cc + 1) * MC].rearrange(
                            "r (l c) -> r l c", l=16),
                        in_=T2[cc * 32 : cc * 32 + 18, :].rearrange(
                            "r (c l) -> r c l", l=16).transpose([0, 2, 1]),
                    )

                nc.scalar.activation(out=OM[:], in_=OM[:], func=AF.Sigmoid)
                nc.vector.tensor_scalar_mul(out=OM[:], in0=OM[:], scalar1=2.0)
                nc.vector.tensor_scalar(out=T0[:], in0=OY[:], scalar1=-1.0, scalar2=1.0,
                                        op0=ALU.mult, op1=ALU.add)
                nc.vector.tensor_scalar(out=T1[:], in0=OX[:], scalar1=-1.0, scalar2=1.0,
                                        op0=ALU.mult, op1=ALU.add)
                for qi, (ya, xa) in enumerate(((T0, T1), (T0, OX), (OY, T1), (OY, OX))):
                    nc.vector.tensor_mul(T2[:], ya[:], xa[:])
                    nc.vector.tensor_mul(T2[:], T2[:], OM[:])
                    for cc in range(NPIX // MC):
                        nc.vector.tensor_copy(
                            out=wq[32 * qi : 32 * qi + 18,
                                   cc * MC : (cc + 1) * MC],
                            in_=T2[cc * 32 : cc * 32 + 18, :],
                        )

                # idx16 (wrapped order) -> idxt via DRAM bounce; all reads are
                # 256B-contiguous descriptors.
                idx_dram = nc.dram_tensor("idx_scratch", [18, NPIX], DT.int16)
                nc.sync.dma_start(out=idx_dram[:], in_=idx16[:])
                for h in range(2):
                    srcv = idx_dram[h * 9 : (h + 1) * 9, :].rearrange(
                        "k (cb p c) -> k cb p c", cb=4, p=16, c=128
                    ).transpose([2, 0, 1, 3])
                    for g in range(4):
                        p0 = h * 64 + g * 16
                        nc.sync.dma_start(
                            out=idxt[p0 : p0 + 16, :].rearrange(
                                "p (k cb c) -> p k cb c", k=KF, cb=4, c=128),
                            in_=srcv,
                        )

        # selector lhsT: sel[32*qi + j, qi*128 + (j//9)*64 + o] = 1 for j<18
        with tc.tile_pool(name="selb", bufs=1) as selb:
            rP = selb.tile([128, 1], DT.float32)
            cC = selb.tile([128, 512], DT.float32)
            t1 = selb.tile([128, 512], DT.float32)
            t2 = selb.tile([128, 512], DT.float32)
            _iotas.append(nc.gpsimd.iota(rP[:], pattern=[[0, 1]], channel_multiplier=1,
                           allow_small_or_imprecise_dtypes=True))
            _iotas.append(nc.gpsimd.iota(cC[:], pattern=[[1, 512]], channel_multiplier=0,
                           allow_small_or_imprecise_dtypes=True))
            # j = r mod 32 ; qi_r = (r - j)/32
            j32 = selb.tile([128, 1], DT.float32)
            qir = selb.tile([128, 1], DT.float32)
            jt = selb.tile([128, 1], DT.float32)
            nc.vector.tensor_scalar(out=qir[:], in0=rP[:], scalar1=31.5, scalar2=None, op0=ALU.is_gt)
            nc.vector.tensor_scalar(out=jt[:], in0=rP[:], scalar1=63.5, scalar2=None, op0=ALU.is_gt)
            nc.vector.tensor_add(qir[:], qir[:], jt[:])
            nc.vector.tensor_scalar(out=jt[:], in0=rP[:], scalar1=95.5, scalar2=None, op0=ALU.is_gt)
            nc.vector.tensor_add(qir[:], qir[:], jt[:])
            nc.vector.tensor_scalar(out=j32[:], in0=qir[:], scalar1=-32.0, scalar2=None, op0=ALU.mult)
            nc.vector.tensor_add(j32[:], j32[:], rP[:])
            # cond1: floor(c/128) == qi_r  -> |c/128 - qi_r - frac| via mod
            t3 = selb.tile([128, 512], DT.float32)
            nc.vector.tensor_scalar(out=t2[:], in0=cC[:], scalar1=127.5, scalar2=None, op0=ALU.is_gt)
            nc.vector.tensor_scalar(out=t3[:], in0=cC[:], scalar1=255.5, scalar2=None, op0=ALU.is_gt)
            nc.vector.tensor_add(t2[:], t2[:], t3[:])
            nc.vector.tensor_scalar(out=t3[:], in0=cC[:], scalar1=383.5, scalar2=None, op0=ALU.is_gt)
            nc.vector.tensor_add(t2[:], t2[:], t3[:])   # floor(c/128)
            nc.vector.tensor_scalar(out=t1[:], in0=t2[:], scalar1=-128.0, scalar2=None, op0=ALU.mult)
            nc.vector.tensor_add(t1[:], t1[:], cC[:])   # c mod 128
            nc.vector.tensor_scalar(out=t2[:], in0=t2[:], scalar1=qir[:], scalar2=None,
                                    op0=ALU.is_equal)
            # cond2: floor((c mod 128)/64) == floor(j/9)  (j<18 -> floor(j/9) in {0,1})
            nc.vector.tensor_scalar(out=t1[:], in0=t1[:], scalar1=63.5, scalar2=None,
                                    op0=ALU.is_gt)             # h(c)
            hj = selb.tile([128, 1], DT.float32)
            nc.vector.tensor_scalar(out=hj[:], in0=j32[:], scalar1=8.5, scalar2=None,
                                    op0=ALU.is_gt)             # j>=9
            nc.vector.tensor_scalar(out=t1[:], in0=t1[:], scalar1=hj[:], scalar2=None,
                                    op0=ALU.is_equal)
            nc.vector.tensor_mul(t2[:], t2[:], t1[:])
            # cond3: j < 18
            j18 = selb.tile([128, 1], DT.float32)
            nc.vector.tensor_scalar(out=j18[:], in0=j32[:], scalar1=31.5, scalar2=None,
                                    op0=ALU.is_lt)
            nc.vector.tensor_scalar(out=t2[:], in0=t2[:], scalar1=j18[:], scalar2=None,
                                    op0=ALU.mult)
            # per-k selectivity: jk = j32 - 9*hj ; sel_k = t2 * (jk == k)
            jkk = selb.tile([128, 1], DT.float32)
            nc.vector.tensor_scalar(out=jkk[:], in0=hj[:], scalar1=-9.0, scalar2=None,
                                    op0=ALU.mult)
            nc.vector.tensor_add(jkk[:], jkk[:], j32[:])
            tk = selb.tile([128, 1], DT.float32)
            for k in range(KF):
                nc.vector.tensor_scalar(out=tk[:], in0=jkk[:], scalar1=float(k),
                                        scalar2=None, op0=ALU.is_equal)
                nc.vector.tensor_scalar(out=sel[:, k * 512 : (k + 1) * 512],
                                        in0=t2[:], scalar1=tk[:, 0:1],
                                        scalar2=None, op0=ALU.mult)

        # wdup + b_dc
        nc.vector.memset(wdup[:], 0.0)
        wdc_v = wdc_ext[:].rearrange("o c kh kw -> c o (kh kw)")
        for k in range(KF):
            for h in range(2):
                nc.gpsimd.dma_start(
                    out=wdup[h * 64 : h * 64 + 64,
                             k * 128 + h * 64 : k * 128 + h * 64 + 64],
                    in_=wdc_v[:, :, k : k + 1].rearrange("c a d -> c (a d)"),
                )
        for h in range(2):
            nc.sync.dma_start(
                out=bdc_t[h * 64 : h * 64 + 64, 0:1],
                in_=bdc_ext[:].rearrange("(o one) -> o one", one=1),
            )

        # ======== phase 4+5 in one pool scope
        with (
            tc.tile_pool(name="qt", bufs=1) as qtp,
            tc.tile_pool(name="g", bufs=2) as gpool,
            tc.tile_pool(name="h", bufs=2) as hpool,
            tc.tile_pool(name="o", bufs=2) as opool,
            tc.tile_pool(name="mp", bufs=4, space="PSUM") as mpsum,
            tc.tile_pool(name="op", bufs=1, space="PSUM") as opsum,
        ):
            _lib = nc.gpsimd.load_library(library_config.ap_gather)
            for _io in _iotas:
                tile.add_dep_helper(_lib.ins, _io.ins, reason="lib load after iotas")
            qtab = qtp.tile([128, NBLK * 4], DT.bfloat16)
            nc.vector.memset(qtab[:], 0.0)
            q4 = qtab[:].rearrange("p (blk q) -> p blk q", q=4)
            for a in range(2):
                for b in range(2):
                    blk0 = (a * 2 + b) * (RY_N * RX_N)
                    for qy in range(2):
                        for qx in range(2):
                            ry_cnt = min((75 - a - qy) // 2 + 1, RY_N)
                            rx0 = 1 if (b + qx) == 0 else 0
                            rx1 = min(RX_N - 1, (130 - b - qx) // 2)
                            rx_cnt = rx1 - rx0 + 1
                            c0 = 2 * rx0 + b + qx - 1
                            src = x3()[:, a + qy : a + qy + 2 * (ry_cnt - 1) + 1 : 2,
                                       c0 : c0 + 2 * (rx_cnt - 1) + 1 : 2]
                            dst3 = q4[:, blk0 + rx0 : blk0 + rx0 + (ry_cnt - 1) * RX_N + rx_cnt,
                                      qy * 2 + qx : qy * 2 + qx + 1]
                            dst = bass.AP(
                                dst3.tensor, dst3.offset,
                                [dst3.ap[0], [RX_N * 4, ry_cnt], [4, rx_cnt]],
                            )
                            nc.vector.tensor_copy(out=dst, in_=src)

            qtab_f32 = qtab[:].bitcast(DT.float32)
            outv = out_ext[:].rearrange("o h w -> o (h w)")

            for cb in range(NPIX // GCHUNK):
                po = opsum.tile([128, GCHUNK], DT.float32, tag="po")
                for k in range(KF):
                    g = gpool.tile([128, GCHUNK * 2], DT.float32, tag="g")
                    idx_sl = idxt[:, k * S16 + cb * (GCHUNK // 16):
                                  k * S16 + (cb + 1) * (GCHUNK // 16)]
                    _ga = nc.gpsimd.ap_gather(
                        g[:], qtab_f32, idx_sl,
                        channels=128, num_elems=NBLK, d=2, num_idxs=GCHUNK,
                    )
                    tile.add_dep_helper(_ga.ins, _lib.ins, reason="gather after lib load")
                    gb = g[:].bitcast(DT.bfloat16).rearrange(
                        "p (n q) -> p n q", q=4
                    )
                    for sub in range(GCHUNK // SUB):
                        col0 = cb * GCHUNK + sub * SUB
                        hts = []
                        for qi in range(4):
                            mq = mpsum.tile([128, SUB], DT.float32, tag="mq")
                            nc.tensor.matmul(
                                out=mq[:],
                                lhsT=sel[:, k * 512 + qi * 128 : k * 512 + (qi + 1) * 128],
                                rhs=wq[:, col0 : col0 + SUB],
                                start=True, stop=True,
                            )
                            ht = hpool.tile([128, SUB], DT.bfloat16, tag=f"ht{qi}")
                            gq = gb[:, sub * SUB : (sub + 1) * SUB,
                                    qi : qi + 1].rearrange("p n one -> p (n one)")
                            nc.vector.tensor_mul(ht[:], mq[:], gq)
                            hts.append(ht)
                        for qi in range(4):
                            nc.tensor.matmul(
                                out=po[:, sub * SUB : (sub + 1) * SUB],
                                lhsT=wdup[:, k * 128 : (k + 1) * 128],
                                rhs=hts[qi][:],
                                start=(k == 0 and qi == 0),
                                stop=(k == KF - 1 and qi == 3),
                            )
                ot = opool.tile([128, GCHUNK], DT.float32, tag="ot")
                nc.vector.tensor_scalar(
                    out=ot[:], in0=po[:], scalar1=bdc_t[:, 0:1],
                    scalar2=None, op0=ALU.add,
                )
                for h in range(2):
                    nc.sync.dma_start(
                        out=outv[:, h * NPIX + cb * GCHUNK:
                                 h * NPIX + (cb + 1) * GCHUNK],
                        in_=ot[h * 64 : h * 64 + 64, :],
                    )


def _build_nc():
    _install_compat()
    nc = bass.Bass()
    x_ext = nc.declare_dram_parameter("x", [C, H, W], DT.float32, isOutput=False)
    wom_ext = nc.declare_dram_parameter("w_om", [3 * KF, C, K, K], DT.float32, isOutput=False)
    bom_ext = nc.declare_dram_parameter("b_om", [3 * KF], DT.float32, isOutput=False)
    wdc_ext = nc.declare_dram_parameter("w_dc", [O, C, K, K], DT.float32, isOutput=False)
    bdc_ext = nc.declare_dram_parameter("b_dc", [O], DT.float32, isOutput=False)
    out_ext = nc.declare_dram_parameter("out", [O, H, W], DT.float32, isOutput=True)
    with tile.TileContext(nc) as tc:
        _emit(nc, tc, x_ext, wom_ext, bom_ext, wdc_ext, bdc_ext, out_ext)
    lower_extended_insts(nc)
    return nc


_NC_CACHE = None


def kernel(**inputs):
    global _NC_CACHE
    x = np.ascontiguousarray(inputs["x"], dtype=np.float32)
    w_om = np.ascontiguousarray(inputs["w_om"], dtype=np.float32)
    b_om = np.ascontiguousarray(inputs["b_om"], dtype=np.float32)
    w_dc = np.ascontiguousarray(inputs["w_dc"], dtype=np.float32)
    b_dc = np.ascontiguousarray(inputs["b_dc"], dtype=np.float32)

    if _NC_CACHE is None:
        _NC_CACHE = _build_nc()
    nc = _NC_CACHE

    in_maps = [
        {"x": x[i], "w_om": w_om, "b_om": b_om, "w_dc": w_dc, "b_dc": b_dc}
        for i in range(NCORES)
    ]
    res = run_bass_kernel_spmd(nc, in_maps, core_ids=list(range(NCORES)))
    return np.stack(
        [np.asarray(res.results[i]["out"]) for i in range(NCORES)]
    ).astype(np.float32)



# revision 15
# speedup vs baseline: 1.4400x; 1.0312x over previous
"""Trainium2 Bass kernel for nn_AdaFeatBlock (modulated deformable-conv block).

Sharding: data-parallel over batch — 8 samples -> 8 NeuronCores, all weights
replicated; each core computes its sample end-to-end, host stacks outputs.

Per-core pipeline (one sample, x [64,128,128]):
  1. x -> bf16 "half-split" padded layout: partition h*64+c = channel c of
     image-half h; free = 76 stored rows (h*64-6 .. h*64+69) x 130 cols
     (-1..128), zero-padded borders.
  2. offset/mask 3x3 conv = 9 shifted matmuls, block-diagonal [128, 54]
     lhsT (both halves at once), PSUM-accumulated. Output row order per
     half: [off_y k0..8 | off_x k0..8 | mask k0..8].
  3. Coordinate math on [18, 8192] tiles (partition = (half, k)): bilinear
     corner weights (mask-modulated, zero outside the image via the
     zero-padded gather table) and 2x2-quad table indices.
  4. Quad gather table Q[128, 10032, 4] bf16: 2x2 pixel blocks at all 4
     row/col parities. ap_gather (d=2 f32 view = 8B quad) fetches a
     point's 4 corners for 16 channels/core; 8 Q7 cores cover
     128 partitions = 2 halves x 64 channels.
  5. Per (k, 512-px block): corner-weight rows broadcast to 128 partitions
     via a tiny selector matmul into PSUM; fused PSUM-read multiply into
     gathered corners; 3 adds -> modulated bilinear sample "val".
  6. Deformable einsum: per k a [128->128, 512] matmul with block-diagonal
     channel-duplicated w_dc, PSUM-accumulated over all 9 k. + b_dc -> out.
"""

import numpy as np

import concourse.bass as bass
import concourse.tile as tile
from concourse import mybir
from concourse.bass_utils import run_bass_kernel_spmd
from concourse import library_config
from concourse.library_overlay import lower_extended_insts
from concourse.vector_clock import ScopedClock

AF = mybir.ActivationFunctionType
ALU = mybir.AluOpType
DT = mybir.dt

B, C, H, W = 8, 64, 128, 128
O = 64
K = 3
KF = 9
NCORES = 8
HALF = H // 2
NPIX = H * W // 2              # 8192 pixels per half
ROWS_ST = 76                   # stored rows per half
PITCH = 130                    # stored cols (-1..128)
RY_N, RX_N = 38, 66
NBLK = 4 * RY_N * RX_N         # 10032
GCHUNK = 2048
SUB = 512
S16 = NPIX // 16               # idx ints per partition per k


def _install_compat():
    """This walrus build accepts at most ONE sync-wait per instruction."""
    if getattr(tile.TileContext, "_adafeat_patched", False):
        return
    _orig_lower = tile.TileContext._lower_ordered_insts

    def _split_waits(nc, ordered):
        for insts in ordered.values():
            new_insts = []
            for inst in insts:
                si = inst.sync_info
                if si is not None and si.on_wait and len(si.on_wait) > 1:
                    waits = list(si.on_wait)
                    for w in waits[:-1]:
                        nop = mybir.InstNoOp(name=f"I-{nc.next_id()}", ins=[], outs=[])
                        nop.engine = inst.engine
                        nop.sync_info = mybir.SyncInfo(on_wait=[w], on_update=[])
                        new_insts.append(nop)
                    inst.sync_info = mybir.SyncInfo(
                        on_wait=[waits[-1]], on_update=list(si.on_update)
                    )
                new_insts.append(inst)
            insts[:] = new_insts

    def _lower_split(self, ordered):
        _split_waits(self.nc, ordered)
        return _orig_lower(self, ordered)

    def _drain_split(self, tick_clock, wait_clock):
        carrier = self.nc.sync.nop(nofuse=True)
        wait_clock.add_sem_waits(
            carrier.ins, ScopedClock({None: tick_clock.global_clock})
        )
        si = carrier.ins.sync_info
        if si is not None and si.on_wait and len(si.on_wait) > 1:
            waits = list(si.on_wait)
            carrier.ins.sync_info = mybir.SyncInfo(
                on_wait=waits[:1], on_update=list(si.on_update)
            )
            for w in waits[1:]:
                extra = self.nc.sync.nop(nofuse=True)
                extra.ins.sync_info = mybir.SyncInfo(on_wait=[w], on_update=[])
        self.nc.sync.drain()
        self.nc.all_engine_barrier()
        popped = self.nc._tile_sem_poison_stack.pop()
        assert popped is self._sem_poison
        self.nc.clear_and_free_semaphores(list(self.sems.allocated().values()))
        self.nc.all_engine_barrier()

    tile.TileContext._lower_ordered_insts = _lower_split
    tile.TileContext._drain_and_barrier = _drain_split
    tile.TileContext._adafeat_patched = True


def _emit(nc, tc, x_ext, wom_ext, bom_ext, wdc_ext, bdc_ext, out_ext):
    _iotas = []

    with tc.tile_pool(name="persist", bufs=1) as persist:
        x_sb = None  # allocated in xpool below
        wq = persist.tile([128, NPIX], DT.bfloat16)
        idxt = persist.tile([128, KF * S16], DT.int16)
        wdup = persist.tile([128, KF * 128], DT.bfloat16)
        sel = persist.tile([128, KF * 4 * 128], DT.bfloat16)
        bdc_t = persist.tile([128, 1], DT.float32)
        qtab = persist.tile([128, NBLK * 4], DT.bfloat16)
        ompool_cm = tc.tile_pool(name="omp", bufs=1)
        ompool = ompool_cm.__enter__()
        om = ompool.tile([54, NPIX], DT.bfloat16)

        xpool = tc.tile_pool(name="xp", bufs=1)
        xp = xpool.__enter__()
        x_sb = xp.tile([128, ROWS_ST * PITCH], DT.bfloat16)

        x3 = lambda: x_sb[:].rearrange("p (r c) -> p r c", c=PITCH)

        # ======== phase 1: load x (f32 -> bf16), half-split, zero-padded
        nc.vector.memset(x_sb[:], 0.0)
        nc.vector.memset(wq[:], 0.0)
        xv = x_ext[:]
        for h in range(2):
            r0 = max(0, h * HALF - 6)
            r1 = min(H - 1, h * HALF + 69)
            nrow = r1 - r0 + 1
            rloc = r0 - (h * HALF - 6)
            dst = x3()[h * 64 : h * 64 + 64, rloc : rloc + nrow, 1 : 1 + W]
            nc.gpsimd.dma_start(out=dst, in_=xv[:, r0 : r1 + 1, :])

        # ======== phase 2: offset/mask conv
        with (
            tc.tile_pool(name="convw", bufs=1) as convw,
            tc.tile_pool(name="convp", bufs=2, space="PSUM") as convp,
        ):
            # w_om views: y/x roles from rows 0..17 (o=2k+r), m role rows 18..26
            wom_yx = wom_ext[:][0:18].rearrange(
                "(o2 r) c kh kw -> c o2 r (kh kw)", r=2
            )
            wom_m = wom_ext[:][18:27].rearrange("o c kh kw -> c o (kh kw)")
            lhs_om = []
            for dy in range(3):
                for dx in range(3):
                    dd = dy * K + dx
                    t = convw.tile([128, 54], DT.bfloat16, tag=f"lom{dd}")
                    nc.vector.memset(t[:], 0.0)
                    for h in range(2):
                        ps = slice(h * 64, h * 64 + 64)
                        nc.gpsimd.dma_start(
                            out=t[ps, h * 27 + 0 : h * 27 + 9],
                            in_=wom_yx[:, 0:9, 0:1, dd : dd + 1].rearrange(
                                "c a b d -> c (a b d)"),
                        )
                        nc.gpsimd.dma_start(
                            out=t[ps, h * 27 + 9 : h * 27 + 18],
                            in_=wom_yx[:, 0:9, 1:2, dd : dd + 1].rearrange(
                                "c a b d -> c (a b d)"),
                        )
                        nc.gpsimd.dma_start(
                            out=t[ps, h * 27 + 18 : h * 27 + 27],
                            in_=wom_m[:, :, dd : dd + 1].rearrange(
                                "c a d -> c (a d)"),
                        )
                    lhs_om.append(t)

            bom_t = convw.tile([54, 1], DT.float32)
            bom_yx = bom_ext[:][0:18].rearrange("(o2 r) -> o2 r", r=2)
            for h in range(2):
                nc.sync.dma_start(
                    out=bom_t[h * 27 + 0 : h * 27 + 9, 0:1], in_=bom_yx[0:9, 0:1]
                )
                nc.sync.dma_start(
                    out=bom_t[h * 27 + 9 : h * 27 + 18, 0:1], in_=bom_yx[0:9, 1:2]
                )
                nc.sync.dma_start(
                    out=bom_t[h * 27 + 18 : h * 27 + 27, 0:1],
                    in_=bom_ext[:][18:27].rearrange("(o one) -> o one", one=1),
                )

            rows_per_sub = SUB // W  # 4
            for cb in range(NPIX // SUB):
                pt = convp.tile([54, SUB], DT.float32, tag="cpt")
                r0 = cb * rows_per_sub
                for i, (dy, dx) in enumerate(
                    (dy, dx) for dy in range(3) for dx in range(3)
                ):
                    rhs = x3()[:, 6 + r0 + dy - 1 : 6 + r0 + dy - 1 + rows_per_sub,
                               dx : dx + W]
                    nc.tensor.matmul(
                        out=pt[:], lhsT=lhs_om[i][:], rhs=rhs,
                        start=(i == 0), stop=(i == 8),
                    )
                nc.vector.tensor_scalar(
                    out=om[:, cb * SUB : (cb + 1) * SUB], in0=pt[:],
                    scalar1=bom_t[:, 0:1], scalar2=None, op0=ALU.add,
                )

            # ======== phase 3: coordinate math, single pass on [72, 2048]
            # partition P = cc*32 + h*9 + k  (cc = 2048-px chunk of the half; 32-aligned blocks)
            with tc.tile_pool(name="math", bufs=1) as mpool:
                idx16b = mpool.tile([128, 2048], DT.int16)
                OY = mpool.tile([128, 2048], DT.float32)
                OX = mpool.tile([128, 2048], DT.float32)
                OM = mpool.tile([128, 2048], DT.float32)
                IOT = mpool.tile([128, 2048], DT.float32)
                T0 = mpool.tile([128, 2048], DT.float32)
                T1 = mpool.tile([128, 2048], DT.float32)
                T2 = mpool.tile([128, 2048], DT.float32)
                T3 = mpool.tile([128, 2048], DT.float32)
                cst = mpool.tile([128, 6], DT.float32)

                pidx = mpool.tile([128, 4], DT.float32)
                _iotas.append(nc.gpsimd.iota(pidx[:, 0:1], pattern=[[0, 1]],
                               channel_multiplier=1,
                               allow_small_or_imprecise_dtypes=True))
                P128 = pidx[:, 0:1]
                hh, kk, kh3, km3, ccv, hkv = (cst[:, i : i + 1] for i in range(6))
                t_a = pidx[:, 1:2]
                # cc = P // 18
                nc.vector.tensor_scalar(out=ccv, in0=P128, scalar1=31.5, scalar2=None, op0=ALU.is_gt)
                nc.vector.tensor_scalar(out=t_a, in0=P128, scalar1=63.5, scalar2=None, op0=ALU.is_gt)
                nc.vector.tensor_add(ccv, ccv, t_a)
                nc.vector.tensor_scalar(out=t_a, in0=P128, scalar1=95.5, scalar2=None, op0=ALU.is_gt)
                nc.vector.tensor_add(ccv, ccv, t_a)
                # hk = P - 18*cc ; h = hk > 8.5 ; k = hk - 9*h
                nc.vector.tensor_scalar(out=hkv, in0=ccv, scalar1=-32.0, scalar2=None, op0=ALU.mult)
                nc.vector.tensor_add(hkv, hkv, P128)
                nc.vector.tensor_scalar(out=hh, in0=hkv, scalar1=8.5, scalar2=None, op0=ALU.is_gt)
                nc.vector.tensor_scalar(out=kk, in0=hh, scalar1=-9.0, scalar2=None, op0=ALU.mult)
                nc.vector.tensor_add(kk, kk, hkv)
                nc.vector.tensor_scalar(out=kh3, in0=kk, scalar1=2.5, scalar2=None, op0=ALU.is_gt)
                nc.vector.tensor_scalar(out=t_a, in0=kk, scalar1=5.5, scalar2=None, op0=ALU.is_gt)
                nc.vector.tensor_add(kh3, kh3, t_a)
                nc.vector.tensor_scalar(out=km3, in0=kh3, scalar1=-3.0, scalar2=None, op0=ALU.mult)
                nc.vector.tensor_add(km3, km3, kk)
                cstv = mpool.tile([128, 4], DT.float32, tag="cstv")
                # cstv0 = 64*h + 511 + kh3 + 16*cc
                nc.vector.tensor_scalar(out=cstv[:, 0:1], in0=hh, scalar1=64.0, scalar2=511.0,
                                        op0=ALU.mult, op1=ALU.add)
                nc.vector.tensor_add(cstv[:, 0:1], cstv[:, 0:1], kh3)
                nc.vector.tensor_scalar(out=t_a, in0=ccv, scalar1=16.0, scalar2=None, op0=ALU.mult)
                nc.vector.tensor_add(cstv[:, 0:1], cstv[:, 0:1], t_a)
                nc.vector.tensor_scalar(out=cstv[:, 1:2], in0=km3, scalar1=511.0, scalar2=None, op0=ALU.add)
                nc.vector.tensor_scalar(out=cstv[:, 2:3], in0=hh, scalar1=-64.0, scalar2=6.0 - 512.0,
                                        op0=ALU.mult, op1=ALU.add)

                MC = 2048
                for cc in range(NPIX // MC):
                    cs = slice(cc * MC, (cc + 1) * MC)
                    for role, dstt in ((0, OY), (1, OX), (2, OM)):
                        for h in range(2):
                            nc.gpsimd.dma_start(
                                out=dstt[cc * 32 + h * 9 : cc * 32 + h * 9 + 9, :],
                                in_=om[h * 27 + role * 9 : h * 27 + role * 9 + 9, cs],
                            )
                # py = OY + rowbase ; fy = mod(py,1); y0f = py - fy
                _iotas.append(nc.gpsimd.iota(IOT[:], pattern=[[1, MC // W], [0, W]],
                               channel_multiplier=0,
                               allow_small_or_imprecise_dtypes=True))
                nc.vector.tensor_add(T0[:], OY[:], IOT[:])
                nc.vector.tensor_scalar(out=T0[:], in0=T0[:], scalar1=cstv[:, 0:1],
                                        scalar2=None, op0=ALU.add)
                nc.vector.tensor_scalar(out=T2[:], in0=T0[:], scalar1=8388608.0, scalar2=-8388608.0,
                                        op0=ALU.add, op1=ALU.add)
                nc.vector.tensor_tensor(out=OY[:], in0=T2[:], in1=T0[:], op=ALU.is_gt)
                nc.vector.tensor_sub(T2[:], T2[:], OY[:])
                nc.vector.tensor_sub(OY[:], T0[:], T2[:])
                nc.vector.tensor_copy(out=T0[:], in_=T2[:])
                _iotas.append(nc.gpsimd.iota(IOT[:], pattern=[[0, MC // W], [1, W]],
                               channel_multiplier=0,
                               allow_small_or_imprecise_dtypes=True))
                nc.vector.tensor_add(T1[:], OX[:], IOT[:])
                nc.vector.tensor_scalar(out=T1[:], in0=T1[:], scalar1=cstv[:, 1:2],
                                        scalar2=None, op0=ALU.add)
                nc.vector.tensor_scalar(out=T2[:], in0=T1[:], scalar1=8388608.0, scalar2=-8388608.0,
                                        op0=ALU.add, op1=ALU.add)
                nc.vector.tensor_tensor(out=OX[:], in0=T2[:], in1=T1[:], op=ALU.is_gt)
                nc.vector.tensor_sub(T2[:], T2[:], OX[:])
                nc.vector.tensor_sub(OX[:], T1[:], T2[:])
                nc.vector.tensor_copy(out=T1[:], in_=T2[:])

                nc.vector.tensor_scalar(out=T0[:], in0=T0[:], scalar1=cstv[:, 2:3],
                                        scalar2=None, op0=ALU.add)
                nc.vector.tensor_scalar(out=T0[:], in0=T0[:], scalar1=0.0, scalar2=75.0,
                                        op0=ALU.max, op1=ALU.min)
                nc.vector.tensor_scalar_mul(out=T0[:], in0=T0[:], scalar1=0.5)
                nc.vector.tensor_scalar(out=T3[:], in0=T0[:], scalar1=8388608.0, scalar2=-8388608.0,
                                        op0=ALU.add, op1=ALU.add)
                nc.vector.tensor_tensor(out=T2[:], in0=T3[:], in1=T0[:], op=ALU.is_gt)
                nc.vector.tensor_sub(T3[:], T3[:], T2[:])
                nc.vector.tensor_sub(T2[:], T0[:], T3[:])
                nc.vector.tensor_copy(out=T0[:], in_=T3[:])
                nc.vector.tensor_scalar(out=T1[:], in0=T1[:], scalar1=2.0 - 512.0,
                                        scalar2=None, op0=ALU.add)
                nc.vector.tensor_scalar(out=T1[:], in0=T1[:], scalar1=0.0, scalar2=130.0,
                                        op0=ALU.max, op1=ALU.min)
                nc.vector.tensor_scalar_mul(out=T1[:], in0=T1[:], scalar1=0.5)
                nc.vector.tensor_scalar(out=IOT[:], in0=T1[:], scalar1=8388608.0, scalar2=-8388608.0,
                                        op0=ALU.add, op1=ALU.add)
                nc.vector.tensor_tensor(out=T3[:], in0=IOT[:], in1=T1[:], op=ALU.is_gt)
                nc.vector.tensor_sub(IOT[:], IOT[:], T3[:])
                nc.vector.tensor_sub(T3[:], T1[:], IOT[:])
                nc.vector.tensor_copy(out=T1[:], in_=IOT[:])

                nc.vector.tensor_scalar_mul(out=T2[:], in0=T2[:], scalar1=float(4 * RY_N * RX_N))
                nc.vector.tensor_scalar_mul(out=T3[:], in0=T3[:], scalar1=float(2 * RY_N * RX_N))
                nc.vector.tensor_add(T2[:], T2[:], T3[:])
                nc.vector.tensor_scalar_mul(out=T0[:], in0=T0[:], scalar1=float(RX_N))
                nc.vector.tensor_add(T2[:], T2[:], T0[:])
                nc.vector.tensor_add(T2[:], T2[:], T1[:])
                # wrapped idx, P-layout rows; bounce each chunk to DRAM
                # idx_dram[h*9+k, cc*2048 + p*128 + c] = idx of px cc*2048+c*16+p
                idx_dram = nc.dram_tensor("idx_scratch", [18, NPIX], DT.int16)
                for cc in range(NPIX // MC):
                    nc.vector.tensor_copy(
                        out=idx16b[cc * 32 : cc * 32 + 18, :].rearrange(
                            "r (l c) -> r l c", l=16),
                        in_=T2[cc * 32 : cc * 32 + 18, :].rearrange(
                            "r (c l) -> r c l", l=16).transpose([0, 2, 1]),
                    )
                    nc.sync.dma_start(
                        out=idx_dram[0:18, cc * MC : (cc + 1) * MC],
                        in_=idx16b[cc * 32 : cc * 32 + 18, :],
                    )

                nc.scalar.activation(out=OM[:], in_=OM[:], func=AF.Sigmoid)
                nc.vector.tensor_scalar_mul(out=OM[:], in0=OM[:], scalar1=2.0)
                nc.vector.tensor_scalar(out=T0[:], in0=OY[:], scalar1=-1.0, scalar2=1.0,
                                        op0=ALU.mult, op1=ALU.add)
                nc.vector.tensor_scalar(out=T1[:], in0=OX[:], scalar1=-1.0, scalar2=1.0,
                                        op0=ALU.mult, op1=ALU.add)
                nc.vector.tensor_mul(T0[:], T0[:], OM[:])   # (1-fy)*m
                nc.vector.tensor_mul(OY[:], OY[:], OM[:])   # fy*m
                for qi, (ya, xa) in enumerate(((T0, T1), (T0, OX), (OY, T1), (OY, OX))):
                    dst_t = T2 if qi % 2 == 0 else T3
                    nc.vector.tensor_mul(dst_t[:], ya[:], xa[:])
                    for cc in range(NPIX // MC):
                        nc.scalar.activation(
                            out=wq[32 * qi : 32 * qi + 18,
                                   cc * MC : (cc + 1) * MC],
                            in_=dst_t[cc * 32 : cc * 32 + 18, :], func=AF.Copy)

                for h in range(2):
                    for cc in range(4):
                        srcv = idx_dram[h * 9 : (h + 1) * 9,
                                        cc * MC : (cc + 1) * MC].rearrange(
                            "k (p c) -> k p c", p=16, c=128
                        ).transpose([1, 0, 2])
                        for g in range(4):
                            p0 = h * 64 + g * 16
                            nc.sync.dma_start(
                                out=idxt[p0 : p0 + 16,
                                         cc * KF * 128 : (cc + 1) * KF * 128
                                         ].rearrange("p (k c) -> p k c", k=KF, c=128),
                                in_=srcv,
                            )

        # selector lhsT: sel[32*qi + j, qi*128 + (j//9)*64 + o] = 1 for j<18
        ompool_cm.__exit__(None, None, None)

        with tc.tile_pool(name="selb", bufs=1) as selb:
            rP = selb.tile([128, 1], DT.float32)
            cC = selb.tile([128, 512], DT.float32)
            t1 = selb.tile([128, 512], DT.float32)
            t2 = selb.tile([128, 512], DT.float32)
            _iotas.append(nc.gpsimd.iota(rP[:], pattern=[[0, 1]], channel_multiplier=1,
                           allow_small_or_imprecise_dtypes=True))
            _iotas.append(nc.gpsimd.iota(cC[:], pattern=[[1, 512]], channel_multiplier=0,
                           allow_small_or_imprecise_dtypes=True))
            # j = r mod 32 ; qi_r = (r - j)/32
            j32 = selb.tile([128, 1], DT.float32)
            qir = selb.tile([128, 1], DT.float32)
            jt = selb.tile([128, 1], DT.float32)
            nc.vector.tensor_scalar(out=qir[:], in0=rP[:], scalar1=31.5, scalar2=None, op0=ALU.is_gt)
            nc.vector.tensor_scalar(out=jt[:], in0=rP[:], scalar1=63.5, scalar2=None, op0=ALU.is_gt)
            nc.vector.tensor_add(qir[:], qir[:], jt[:])
            nc.vector.tensor_scalar(out=jt[:], in0=rP[:], scalar1=95.5, scalar2=None, op0=ALU.is_gt)
            nc.vector.tensor_add(qir[:], qir[:], jt[:])
            nc.vector.tensor_scalar(out=j32[:], in0=qir[:], scalar1=-32.0, scalar2=None, op0=ALU.mult)
            nc.vector.tensor_add(j32[:], j32[:], rP[:])
            # cond1: floor(c/128) == qi_r  -> |c/128 - qi_r - frac| via mod
            t3 = selb.tile([128, 512], DT.float32)
            nc.vector.tensor_scalar(out=t2[:], in0=cC[:], scalar1=127.5, scalar2=None, op0=ALU.is_gt)
            nc.vector.tensor_scalar(out=t3[:], in0=cC[:], scalar1=255.5, scalar2=None, op0=ALU.is_gt)
            nc.vector.tensor_add(t2[:], t2[:], t3[:])
            nc.vector.tensor_scalar(out=t3[:], in0=cC[:], scalar1=383.5, scalar2=None, op0=ALU.is_gt)
            nc.vector.tensor_add(t2[:], t2[:], t3[:])   # floor(c/128)
            nc.vector.tensor_scalar(out=t1[:], in0=t2[:], scalar1=-128.0, scalar2=None, op0=ALU.mult)
            nc.vector.tensor_add(t1[:], t1[:], cC[:])   # c mod 128
            nc.vector.tensor_scalar(out=t2[:], in0=t2[:], scalar1=qir[:], scalar2=None,
                                    op0=ALU.is_equal)
            # cond2: floor((c mod 128)/64) == floor(j/9)  (j<18 -> floor(j/9) in {0,1})
            nc.vector.tensor_scalar(out=t1[:], in0=t1[:], scalar1=63.5, scalar2=None,
                                    op0=ALU.is_gt)             # h(c)
            hj = selb.tile([128, 1], DT.float32)
            nc.vector.tensor_scalar(out=hj[:], in0=j32[:], scalar1=8.5, scalar2=None,
                                    op0=ALU.is_gt)             # j>=9
            nc.vector.tensor_scalar(out=t1[:], in0=t1[:], scalar1=hj[:], scalar2=None,
                                    op0=ALU.is_equal)
            nc.vector.tensor_mul(t2[:], t2[:], t1[:])
            # cond3: j < 18
            j18 = selb.tile([128, 1], DT.float32)
            nc.vector.tensor_scalar(out=j18[:], in0=j32[:], scalar1=31.5, scalar2=None,
                                    op0=ALU.is_lt)
            nc.vector.tensor_scalar(out=t2[:], in0=t2[:], scalar1=j18[:], scalar2=None,
                                    op0=ALU.mult)
            # per-k selectivity: jk = j32 - 9*hj ; sel_k = t2 * (jk == k)
            jkk = selb.tile([128, 1], DT.float32)
            nc.vector.tensor_scalar(out=jkk[:], in0=hj[:], scalar1=-9.0, scalar2=None,
                                    op0=ALU.mult)
            nc.vector.tensor_add(jkk[:], jkk[:], j32[:])
            tk = selb.tile([128, 1], DT.float32)
            for k in range(KF):
                nc.vector.tensor_scalar(out=tk[:], in0=jkk[:], scalar1=float(k),
                                        scalar2=None, op0=ALU.is_equal)
                nc.vector.tensor_scalar(out=sel[:, k * 512 : (k + 1) * 512],
                                        in0=t2[:], scalar1=tk[:, 0:1],
                                        scalar2=None, op0=ALU.mult)

        # wdup + b_dc
        nc.vector.memset(wdup[:], 0.0)
        wdc_v = wdc_ext[:].rearrange("o c kh kw -> c o (kh kw)")
        for k in range(KF):
            for h in range(2):
                nc.gpsimd.dma_start(
                    out=wdup[h * 64 : h * 64 + 64,
                             k * 128 + h * 64 : k * 128 + h * 64 + 64],
                    in_=wdc_v[:, :, k : k + 1].rearrange("c a d -> c (a d)"),
                )
        for h in range(2):
            nc.sync.dma_start(
                out=bdc_t[h * 64 : h * 64 + 64, 0:1],
                in_=bdc_ext[:].rearrange("(o one) -> o one", one=1),
            )

        # ======== quad table build (needs x_sb; before xpool closes)
        nc.gpsimd.memset(qtab[:], 0.0)
        q4 = qtab[:].rearrange("p (blk q) -> p blk q", q=4)
        for a in range(2):
            for b in range(2):
                blk0 = (a * 2 + b) * (RY_N * RX_N)
                for qy in range(2):
                    for qx in range(2):
                        ry_cnt = min((75 - a - qy) // 2 + 1, RY_N)
                        rx0 = 1 if (b + qx) == 0 else 0
                        rx1 = min(RX_N - 1, (130 - b - qx) // 2)
                        rx_cnt = rx1 - rx0 + 1
                        c0 = 2 * rx0 + b + qx - 1
                        src = x3()[:, a + qy : a + qy + 2 * (ry_cnt - 1) + 1 : 2,
                                   c0 : c0 + 2 * (rx_cnt - 1) + 1 : 2]
                        dst3 = q4[:, blk0 + rx0 : blk0 + rx0 + (ry_cnt - 1) * RX_N + rx_cnt,
                                  qy * 2 + qx : qy * 2 + qx + 1]
                        dst = bass.AP(
                            dst3.tensor, dst3.offset,
                            [dst3.ap[0], [RX_N * 4, ry_cnt], [4, rx_cnt]],
                        )
                        nc.scalar.activation(out=dst, in_=src, func=AF.Copy)

        xpool.__exit__(None, None, None)

        # ======== phase 4+5: gathers (k-pairs) + modulate + matmul
        with (
            tc.tile_pool(name="g", bufs=2) as gpool,
            tc.tile_pool(name="h", bufs=2) as hpool,
            tc.tile_pool(name="o", bufs=2) as opool,
            tc.tile_pool(name="mp", bufs=4, space="PSUM") as mpsum,
            tc.tile_pool(name="op", bufs=1, space="PSUM") as opsum,
        ):
            _lib = nc.gpsimd.load_library(library_config.ap_gather)
            for _io in _iotas:
                tile.add_dep_helper(_lib.ins, _io.ins, reason="lib load after iotas")
            qtab_f32 = qtab[:].bitcast(DT.float32)
            outv = out_ext[:].rearrange("o h w -> o (h w)")

            KP = [(0, 1), (2, 3), (4, 5), (6, 7), (8,)]
            for cb in range(NPIX // GCHUNK):
                po = opsum.tile([128, GCHUNK], DT.float32, tag="po")
                for kp in KP:
                    nk = len(kp)
                    g = gpool.tile([128, 2 * GCHUNK * 2], DT.float32, tag="g")
                    idx_sl = idxt[:, (cb * KF + kp[0]) * 128 :
                                  (cb * KF + kp[0] + nk) * 128]
                    _ga = nc.gpsimd.ap_gather(
                        g[:, 0 : nk * GCHUNK * 2], qtab_f32, idx_sl,
                        channels=128, num_elems=NBLK, d=2, num_idxs=nk * GCHUNK,
                    )
                    tile.add_dep_helper(_ga.ins, _lib.ins, reason="gather after lib load")
                    gb = g[:].bitcast(DT.bfloat16).rearrange(
                        "p (n q) -> p n q", q=4
                    )
                    for ki, k in enumerate(kp):
                        for sub in range(GCHUNK // SUB):
                            col0 = cb * GCHUNK + sub * SUB
                            hts = []
                            for qi in range(4):
                                mq = mpsum.tile([128, SUB], DT.float32, tag="mq")
                                nc.tensor.matmul(
                                    out=mq[:],
                                    lhsT=sel[:, k * 512 + qi * 128 : k * 512 + (qi + 1) * 128],
                                    rhs=wq[:, col0 : col0 + SUB],
                                    start=True, stop=True,
                                )
                                ht = hpool.tile([128, SUB], DT.bfloat16, tag=f"ht{qi}")
                                gq = gb[:, ki * GCHUNK + sub * SUB :
                                        ki * GCHUNK + (sub + 1) * SUB,
                                        qi : qi + 1].rearrange("p n one -> p (n one)")
                                nc.vector.tensor_mul(ht[:], mq[:], gq)
                                hts.append(ht)
                            for qi in range(4):
                                nc.tensor.matmul(
                                    out=po[:, sub * SUB : (sub + 1) * SUB],
                                    lhsT=wdup[:, k * 128 : (k + 1) * 128],
                                    rhs=hts[qi][:],
                                    start=(k == 0 and qi == 0),
                                    stop=(k == KF - 1 and qi == 3),
                                )
                ot = opool.tile([128, GCHUNK], DT.float32, tag="ot")
                nc.vector.tensor_scalar(
                    out=ot[:], in0=po[:], scalar1=bdc_t[:, 0:1],
                    scalar2=None, op0=ALU.add,
                )
                for h in range(2):
                    nc.sync.dma_start(
                        out=outv[:, h * NPIX + cb * GCHUNK:
                                 h * NPIX + (cb + 1) * GCHUNK],
                        in_=ot[h * 64 : h * 64 + 64, :],
                    )


def _build_nc():
    _install_compat()
    nc = bass.Bass()
    x_ext = nc.declare_dram_parameter("x", [C, H, W], DT.float32, isOutput=False)
    wom_ext = nc.declare_dram_parameter("w_om", [3 * KF, C, K, K], DT.float32, isOutput=False)
    bom_ext = nc.declare_dram_parameter("b_om", [3 * KF], DT.float32, isOutput=False)
    wdc_ext = nc.declare_dram_parameter("w_dc", [O, C, K, K], DT.float32, isOutput=False)
    bdc_ext = nc.declare_dram_parameter("b_dc", [O], DT.float32, isOutput=False)
    out_ext = nc.declare_dram_parameter("out", [O, H, W], DT.float32, isOutput=True)
    with tile.TileContext(nc) as tc:
        _emit(nc, tc, x_ext, wom_ext, bom_ext, wdc_ext, bdc_ext, out_ext)
    lower_extended_insts(nc)
    return nc


_NC_CACHE = None


def kernel(**inputs):
    global _NC_CACHE
    x = np.ascontiguousarray(inputs["x"], dtype=np.float32)
    w_om = np.ascontiguousarray(inputs["w_om"], dtype=np.float32)
    b_om = np.ascontiguousarray(inputs["b_om"], dtype=np.float32)
    w_dc = np.ascontiguousarray(inputs["w_dc"], dtype=np.float32)
    b_dc = np.ascontiguousarray(inputs["b_dc"], dtype=np.float32)

    if _NC_CACHE is None:
        _NC_CACHE = _build_nc()
    nc = _NC_CACHE

    in_maps = [
        {"x": x[i], "w_om": w_om, "b_om": b_om, "w_dc": w_dc, "b_dc": b_dc}
        for i in range(NCORES)
    ]
    res = run_bass_kernel_spmd(nc, in_maps, core_ids=list(range(NCORES)))
    return np.stack(
        [np.asarray(res.results[i]["out"]) for i in range(NCORES)]
    ).astype(np.float32)



# revision 16
# speedup vs baseline: 1.4401x; 1.0000x over previous
"""Trainium2 Bass kernel for nn_AdaFeatBlock (modulated deformable-conv block).

Sharding: data-parallel over batch — 8 samples -> 8 NeuronCores, all weights
replicated; each core computes its sample end-to-end, host stacks outputs.

Per-core pipeline (one sample, x [64,128,128]):
  1. x -> bf16 "half-split" padded layout: partition h*64+c = channel c of
     image-half h; free = 76 stored rows (h*64-6 .. h*64+69) x 130 cols
     (-1..128), zero-padded borders (scoped pool, freed before gathers).
  2. offset/mask 3x3 conv = 9 shifted matmuls, block-diagonal [128, 54]
     lhsT (both halves at once), PSUM-accumulated. Output row order per
     half: [off_y k0..8 | off_x k0..8 | mask k0..8].
  3. Quad gather table Q[128, 10032, 4] bf16 (persist): 2x2 pixel blocks
     at all 4 row/col parities; memset on gpsimd, fill copies on the
     scalar engine (keeps the DVE free for coordinate math).
  4. Coordinate math in ONE pass on [128, 2048] tiles, partition
     P = 32*chunk + h*9 + k (32-aligned blocks; 72 of 128 rows used):
     floor/frac via the 2^23 trick, mask-modulated corner weights
     (products on DVE, wq row copies on the scalar engine), and quad
     indices written directly in ap_gather's WRAPPED column order
     (in-partition strided copy), so the DRAM bounce and the idxt
     readback DMAs use 256B-contiguous descriptors (the naive transposed
     broadcast was ~590K 2-byte descriptors = 1.3 ms of DMA).
  5. Gathers run as k-PAIRS: ap_gather with num_idxs=4096 (two taps of a
     2048-px chunk per call, idxt laid out (chunk, k)-major) to amortize
     the ~10 us fixed per-call Q7 overhead; 20 calls instead of 36.
  6. Per (k, 512-px block): corner-weight rows broadcast to 128 partitions
     via a selector matmul into PSUM; PSUM-read multiply into gathered
     corners; deformable einsum = [128->128, 512] matmuls with
     block-diagonal channel-duplicated w_dc, PSUM-accumulated over
     (9 k x 4 corners). + b_dc -> out.

Bottleneck: the Q7 ap_gather at ~27 ns/idx (73,728 idx/core ~ 2.0 ms).
SWDGE dma_gather (InstDMAGatherAnt) would move this to the DMA engines but
crashes this build (walrus reports "DynamicDMA is disabled"); fp8 quads
only save ~4% (per-idx bound, not byte bound) and break the 2e-2 rel-err
budget (2.7e-2).
"""

import numpy as np

import concourse.bass as bass
import concourse.tile as tile
from concourse import mybir
from concourse.bass_utils import run_bass_kernel_spmd
from concourse import library_config
from concourse.library_overlay import lower_extended_insts
from concourse.vector_clock import ScopedClock

AF = mybir.ActivationFunctionType
ALU = mybir.AluOpType
DT = mybir.dt

B, C, H, W = 8, 64, 128, 128
O = 64
K = 3
KF = 9
NCORES = 8
HALF = H // 2
NPIX = H * W // 2              # 8192 pixels per half
ROWS_ST = 76                   # stored rows per half
PITCH = 130                    # stored cols (-1..128)
RY_N, RX_N = 38, 66
NBLK = 4 * RY_N * RX_N         # 10032
GCHUNK = 2048
SUB = 512
S16 = NPIX // 16               # idx ints per partition per k


def _install_compat():
    """This walrus build accepts at most ONE sync-wait per instruction."""
    if getattr(tile.TileContext, "_adafeat_patched", False):
        return
    _orig_lower = tile.TileContext._lower_ordered_insts

    def _split_waits(nc, ordered):
        for insts in ordered.values():
            new_insts = []
            for inst in insts:
                si = inst.sync_info
                if si is not None and si.on_wait and len(si.on_wait) > 1:
                    waits = list(si.on_wait)
                    for w in waits[:-1]:
                        nop = mybir.InstNoOp(name=f"I-{nc.next_id()}", ins=[], outs=[])
                        nop.engine = inst.engine
                        nop.sync_info = mybir.SyncInfo(on_wait=[w], on_update=[])
                        new_insts.append(nop)
                    inst.sync_info = mybir.SyncInfo(
                        on_wait=[waits[-1]], on_update=list(si.on_update)
                    )
                new_insts.append(inst)
            insts[:] = new_insts

    def _lower_split(self, ordered):
        _split_waits(self.nc, ordered)
        return _orig_lower(self, ordered)

    def _drain_split(self, tick_clock, wait_clock):
        carrier = self.nc.sync.nop(nofuse=True)
        wait_clock.add_sem_waits(
            carrier.ins, ScopedClock({None: tick_clock.global_clock})
        )
        si = carrier.ins.sync_info
        if si is not None and si.on_wait and len(si.on_wait) > 1:
            waits = list(si.on_wait)
            carrier.ins.sync_info = mybir.SyncInfo(
                on_wait=waits[:1], on_update=list(si.on_update)
            )
            for w in waits[1:]:
                extra = self.nc.sync.nop(nofuse=True)
                extra.ins.sync_info = mybir.SyncInfo(on_wait=[w], on_update=[])
        self.nc.sync.drain()
        self.nc.all_engine_barrier()
        popped = self.nc._tile_sem_poison_stack.pop()
        assert popped is self._sem_poison
        self.nc.clear_and_free_semaphores(list(self.sems.allocated().values()))
        self.nc.all_engine_barrier()

    tile.TileContext._lower_ordered_insts = _lower_split
    tile.TileContext._drain_and_barrier = _drain_split
    tile.TileContext._adafeat_patched = True


def _emit(nc, tc, x_ext, wom_ext, bom_ext, wdc_ext, bdc_ext, out_ext):
    _iotas = []

    with tc.tile_pool(name="persist", bufs=1) as persist:
        x_sb = None  # allocated in xpool below
        wq = persist.tile([128, NPIX], DT.bfloat16)
        idxt = persist.tile([128, KF * S16], DT.int16)
        wdup = persist.tile([128, KF * 128], DT.bfloat16)
        sel = persist.tile([128, KF * 4 * 128], DT.bfloat16)
        bdc_t = persist.tile([128, 1], DT.float32)
        qtab = persist.tile([128, NBLK * 4], DT.bfloat16)
        ompool_cm = tc.tile_pool(name="omp", bufs=1)
        ompool = ompool_cm.__enter__()
        om = ompool.tile([54, NPIX], DT.bfloat16)

        xpool = tc.tile_pool(name="xp", bufs=1)
        xp = xpool.__enter__()
        x_sb = xp.tile([128, ROWS_ST * PITCH], DT.bfloat16)

        x3 = lambda: x_sb[:].rearrange("p (r c) -> p r c", c=PITCH)

        # ======== phase 1: load x (f32 -> bf16), half-split, zero-padded
        nc.vector.memset(x_sb[:], 0.0)
        nc.vector.memset(wq[:], 0.0)
        xv = x_ext[:]
        for h in range(2):
            r0 = max(0, h * HALF - 6)
            r1 = min(H - 1, h * HALF + 69)
            nrow = r1 - r0 + 1
            rloc = r0 - (h * HALF - 6)
            dst = x3()[h * 64 : h * 64 + 64, rloc : rloc + nrow, 1 : 1 + W]
            nc.gpsimd.dma_start(out=dst, in_=xv[:, r0 : r1 + 1, :])

        # ======== phase 2: offset/mask conv
        with (
            tc.tile_pool(name="convw", bufs=1) as convw,
            tc.tile_pool(name="convp", bufs=2, space="PSUM") as convp,
        ):
            # w_om views: y/x roles from rows 0..17 (o=2k+r), m role rows 18..26
            wom_yx = wom_ext[:][0:18].rearrange(
                "(o2 r) c kh kw -> c o2 r (kh kw)", r=2
            )
            wom_m = wom_ext[:][18:27].rearrange("o c kh kw -> c o (kh kw)")
            lhs_om = []
            for dy in range(3):
                for dx in range(3):
                    dd = dy * K + dx
                    t = convw.tile([128, 54], DT.bfloat16, tag=f"lom{dd}")
                    nc.vector.memset(t[:], 0.0)
                    for h in range(2):
                        ps = slice(h * 64, h * 64 + 64)
                        nc.gpsimd.dma_start(
                            out=t[ps, h * 27 + 0 : h * 27 + 9],
                            in_=wom_yx[:, 0:9, 0:1, dd : dd + 1].rearrange(
                                "c a b d -> c (a b d)"),
                        )
                        nc.gpsimd.dma_start(
                            out=t[ps, h * 27 + 9 : h * 27 + 18],
                            in_=wom_yx[:, 0:9, 1:2, dd : dd + 1].rearrange(
                                "c a b d -> c (a b d)"),
                        )
                        nc.gpsimd.dma_start(
                            out=t[ps, h * 27 + 18 : h * 27 + 27],
                            in_=wom_m[:, :, dd : dd + 1].rearrange(
                                "c a d -> c (a d)"),
                        )
                    lhs_om.append(t)

            bom_t = convw.tile([54, 1], DT.float32)
            bom_yx = bom_ext[:][0:18].rearrange("(o2 r) -> o2 r", r=2)
            for h in range(2):
                nc.sync.dma_start(
                    out=bom_t[h * 27 + 0 : h * 27 + 9, 0:1], in_=bom_yx[0:9, 0:1]
                )
                nc.sync.dma_start(
                    out=bom_t[h * 27 + 9 : h * 27 + 18, 0:1], in_=bom_yx[0:9, 1:2]
                )
                nc.sync.dma_start(
                    out=bom_t[h * 27 + 18 : h * 27 + 27, 0:1],
                    in_=bom_ext[:][18:27].rearrange("(o one) -> o one", one=1),
                )

            rows_per_sub = SUB // W  # 4
            for cb in range(NPIX // SUB):
                pt = convp.tile([54, SUB], DT.float32, tag="cpt")
                r0 = cb * rows_per_sub
                for i, (dy, dx) in enumerate(
                    (dy, dx) for dy in range(3) for dx in range(3)
                ):
                    rhs = x3()[:, 6 + r0 + dy - 1 : 6 + r0 + dy - 1 + rows_per_sub,
                               dx : dx + W]
                    nc.tensor.matmul(
                        out=pt[:], lhsT=lhs_om[i][:], rhs=rhs,
                        start=(i == 0), stop=(i == 8),
                    )
                nc.vector.tensor_scalar(
                    out=om[:, cb * SUB : (cb + 1) * SUB], in0=pt[:],
                    scalar1=bom_t[:, 0:1], scalar2=None, op0=ALU.add,
                )

            # ======== phase 3: coordinate math, single pass on [72, 2048]
            # partition P = cc*32 + h*9 + k  (cc = 2048-px chunk of the half; 32-aligned blocks)
            with tc.tile_pool(name="math", bufs=1) as mpool:
                idx16b = mpool.tile([128, 2048], DT.int16)
                OY = mpool.tile([128, 2048], DT.float32)
                OX = mpool.tile([128, 2048], DT.float32)
                OM = mpool.tile([128, 2048], DT.float32)
                IOT = mpool.tile([128, 2048], DT.float32)
                T0 = mpool.tile([128, 2048], DT.float32)
                T1 = mpool.tile([128, 2048], DT.float32)
                T2 = mpool.tile([128, 2048], DT.float32)
                T3 = mpool.tile([128, 2048], DT.float32)
                cst = mpool.tile([128, 6], DT.float32)

                pidx = mpool.tile([128, 4], DT.float32)
                _iotas.append(nc.gpsimd.iota(pidx[:, 0:1], pattern=[[0, 1]],
                               channel_multiplier=1,
                               allow_small_or_imprecise_dtypes=True))
                P128 = pidx[:, 0:1]
                hh, kk, kh3, km3, ccv, hkv = (cst[:, i : i + 1] for i in range(6))
                t_a = pidx[:, 1:2]
                # cc = P // 18
                nc.vector.tensor_scalar(out=ccv, in0=P128, scalar1=31.5, scalar2=None, op0=ALU.is_gt)
                nc.vector.tensor_scalar(out=t_a, in0=P128, scalar1=63.5, scalar2=None, op0=ALU.is_gt)
                nc.vector.tensor_add(ccv, ccv, t_a)
                nc.vector.tensor_scalar(out=t_a, in0=P128, scalar1=95.5, scalar2=None, op0=ALU.is_gt)
                nc.vector.tensor_add(ccv, ccv, t_a)
                # hk = P - 18*cc ; h = hk > 8.5 ; k = hk - 9*h
                nc.vector.tensor_scalar(out=hkv, in0=ccv, scalar1=-32.0, scalar2=None, op0=ALU.mult)
                nc.vector.tensor_add(hkv, hkv, P128)
                nc.vector.tensor_scalar(out=hh, in0=hkv, scalar1=8.5, scalar2=None, op0=ALU.is_gt)
                nc.vector.tensor_scalar(out=kk, in0=hh, scalar1=-9.0, scalar2=None, op0=ALU.mult)
                nc.vector.tensor_add(kk, kk, hkv)
                nc.vector.tensor_scalar(out=kh3, in0=kk, scalar1=2.5, scalar2=None, op0=ALU.is_gt)
                nc.vector.tensor_scalar(out=t_a, in0=kk, scalar1=5.5, scalar2=None, op0=ALU.is_gt)
                nc.vector.tensor_add(kh3, kh3, t_a)
                nc.vector.tensor_scalar(out=km3, in0=kh3, scalar1=-3.0, scalar2=None, op0=ALU.mult)
                nc.vector.tensor_add(km3, km3, kk)
                cstv = mpool.tile([128, 4], DT.float32, tag="cstv")
                # cstv0 = 64*h + 511 + kh3 + 16*cc
                nc.vector.tensor_scalar(out=cstv[:, 0:1], in0=hh, scalar1=64.0, scalar2=511.0,
                                        op0=ALU.mult, op1=ALU.add)
                nc.vector.tensor_add(cstv[:, 0:1], cstv[:, 0:1], kh3)
                nc.vector.tensor_scalar(out=t_a, in0=ccv, scalar1=16.0, scalar2=None, op0=ALU.mult)
                nc.vector.tensor_add(cstv[:, 0:1], cstv[:, 0:1], t_a)
                nc.vector.tensor_scalar(out=cstv[:, 1:2], in0=km3, scalar1=511.0, scalar2=None, op0=ALU.add)
                nc.vector.tensor_scalar(out=cstv[:, 2:3], in0=hh, scalar1=-64.0, scalar2=6.0 - 512.0,
                                        op0=ALU.mult, op1=ALU.add)

                MC = 2048
                for cc in range(NPIX // MC):
                    cs = slice(cc * MC, (cc + 1) * MC)
                    for role, dstt in ((0, OY), (1, OX), (2, OM)):
                        for h in range(2):
                            nc.gpsimd.dma_start(
                                out=dstt[cc * 32 + h * 9 : cc * 32 + h * 9 + 9, :],
                                in_=om[h * 27 + role * 9 : h * 27 + role * 9 + 9, cs],
                            )
                # py = OY + rowbase ; fy = mod(py,1); y0f = py - fy
                _iotas.append(nc.gpsimd.iota(IOT[:], pattern=[[1, MC // W], [0, W]],
                               channel_multiplier=0,
                               allow_small_or_imprecise_dtypes=True))
                nc.vector.tensor_add(T0[:], OY[:], IOT[:])
                nc.vector.tensor_scalar(out=T0[:], in0=T0[:], scalar1=cstv[:, 0:1],
                                        scalar2=None, op0=ALU.add)
                nc.vector.tensor_scalar(out=T2[:], in0=T0[:], scalar1=8388608.0, scalar2=-8388608.0,
                                        op0=ALU.add, op1=ALU.add)
                nc.vector.tensor_tensor(out=OY[:], in0=T2[:], in1=T0[:], op=ALU.is_gt)
                nc.vector.tensor_sub(T2[:], T2[:], OY[:])
                nc.vector.tensor_sub(OY[:], T0[:], T2[:])
                nc.vector.tensor_copy(out=T0[:], in_=T2[:])
                _iotas.append(nc.gpsimd.iota(IOT[:], pattern=[[0, MC // W], [1, W]],
                               channel_multiplier=0,
                               allow_small_or_imprecise_dtypes=True))
                nc.vector.tensor_add(T1[:], OX[:], IOT[:])
                nc.vector.tensor_scalar(out=T1[:], in0=T1[:], scalar1=cstv[:, 1:2],
                                        scalar2=None, op0=ALU.add)
                nc.vector.tensor_scalar(out=T2[:], in0=T1[:], scalar1=8388608.0, scalar2=-8388608.0,
                                        op0=ALU.add, op1=ALU.add)
                nc.vector.tensor_tensor(out=OX[:], in0=T2[:], in1=T1[:], op=ALU.is_gt)
                nc.vector.tensor_sub(T2[:], T2[:], OX[:])
                nc.vector.tensor_sub(OX[:], T1[:], T2[:])
                nc.vector.tensor_copy(out=T1[:], in_=T2[:])

                nc.vector.tensor_scalar(out=T0[:], in0=T0[:], scalar1=cstv[:, 2:3],
                                        scalar2=None, op0=ALU.add)
                nc.vector.tensor_scalar(out=T0[:], in0=T0[:], scalar1=0.0, scalar2=75.0,
                                        op0=ALU.max, op1=ALU.min)
                nc.vector.tensor_scalar_mul(out=T0[:], in0=T0[:], scalar1=0.5)
                nc.vector.tensor_scalar(out=T3[:], in0=T0[:], scalar1=8388608.0, scalar2=-8388608.0,
                                        op0=ALU.add, op1=ALU.add)
                nc.vector.tensor_tensor(out=T2[:], in0=T3[:], in1=T0[:], op=ALU.is_gt)
                nc.vector.tensor_sub(T3[:], T3[:], T2[:])
                nc.vector.tensor_sub(T2[:], T0[:], T3[:])
                nc.vector.tensor_copy(out=T0[:], in_=T3[:])
                nc.vector.tensor_scalar(out=T1[:], in0=T1[:], scalar1=2.0 - 512.0,
                                        scalar2=None, op0=ALU.add)
                nc.vector.tensor_scalar(out=T1[:], in0=T1[:], scalar1=0.0, scalar2=130.0,
                                        op0=ALU.max, op1=ALU.min)
                nc.vector.tensor_scalar_mul(out=T1[:], in0=T1[:], scalar1=0.5)
                nc.vector.tensor_scalar(out=IOT[:], in0=T1[:], scalar1=8388608.0, scalar2=-8388608.0,
                                        op0=ALU.add, op1=ALU.add)
                nc.vector.tensor_tensor(out=T3[:], in0=IOT[:], in1=T1[:], op=ALU.is_gt)
                nc.vector.tensor_sub(IOT[:], IOT[:], T3[:])
                nc.vector.tensor_sub(T3[:], T1[:], IOT[:])
                nc.vector.tensor_copy(out=T1[:], in_=IOT[:])

                nc.vector.tensor_scalar_mul(out=T2[:], in0=T2[:], scalar1=float(4 * RY_N * RX_N))
                nc.vector.tensor_scalar_mul(out=T3[:], in0=T3[:], scalar1=float(2 * RY_N * RX_N))
                nc.vector.tensor_add(T2[:], T2[:], T3[:])
                nc.vector.tensor_scalar_mul(out=T0[:], in0=T0[:], scalar1=float(RX_N))
                nc.vector.tensor_add(T2[:], T2[:], T0[:])
                nc.vector.tensor_add(T2[:], T2[:], T1[:])
                # wrapped idx, P-layout rows; bounce each chunk to DRAM
                # idx_dram[h*9+k, cc*2048 + p*128 + c] = idx of px cc*2048+c*16+p
                idx_dram = nc.dram_tensor("idx_scratch", [18, NPIX], DT.int16)
                for cc in range(NPIX // MC):
                    nc.vector.tensor_copy(
                        out=idx16b[cc * 32 : cc * 32 + 18, :].rearrange(
                            "r (l c) -> r l c", l=16),
                        in_=T2[cc * 32 : cc * 32 + 18, :].rearrange(
                            "r (c l) -> r c l", l=16).transpose([0, 2, 1]),
                    )
                    nc.sync.dma_start(
                        out=idx_dram[0:18, cc * MC : (cc + 1) * MC],
                        in_=idx16b[cc * 32 : cc * 32 + 18, :],
                    )

                nc.scalar.activation(out=OM[:], in_=OM[:], func=AF.Sigmoid)
                nc.vector.tensor_scalar_mul(out=OM[:], in0=OM[:], scalar1=2.0)
                nc.vector.tensor_scalar(out=T0[:], in0=OY[:], scalar1=-1.0, scalar2=1.0,
                                        op0=ALU.mult, op1=ALU.add)
                nc.vector.tensor_scalar(out=T1[:], in0=OX[:], scalar1=-1.0, scalar2=1.0,
                                        op0=ALU.mult, op1=ALU.add)
                nc.vector.tensor_mul(T0[:], T0[:], OM[:])   # (1-fy)*m
                nc.vector.tensor_mul(OY[:], OY[:], OM[:])   # fy*m
                for qi, (ya, xa) in enumerate(((T0, T1), (T0, OX), (OY, T1), (OY, OX))):
                    dst_t = T2 if qi % 2 == 0 else T3
                    nc.vector.tensor_mul(dst_t[:], ya[:], xa[:])
                    for cc in range(NPIX // MC):
                        nc.scalar.activation(
                            out=wq[32 * qi : 32 * qi + 18,
                                   cc * MC : (cc + 1) * MC],
                            in_=dst_t[cc * 32 : cc * 32 + 18, :], func=AF.Copy)

                for h in range(2):
                    for cc in range(4):
                        srcv = idx_dram[h * 9 : (h + 1) * 9,
                                        cc * MC : (cc + 1) * MC].rearrange(
                            "k (p c) -> k p c", p=16, c=128
                        ).transpose([1, 0, 2])
                        for g in range(4):
                            p0 = h * 64 + g * 16
                            nc.sync.dma_start(
                                out=idxt[p0 : p0 + 16,
                                         cc * KF * 128 : (cc + 1) * KF * 128
                                         ].rearrange("p (k c) -> p k c", k=KF, c=128),
                                in_=srcv,
                            )

        # selector lhsT: sel[32*qi + j, qi*128 + (j//9)*64 + o] = 1 for j<18
        ompool_cm.__exit__(None, None, None)

        with tc.tile_pool(name="selb", bufs=1) as selb:
            rP = selb.tile([128, 1], DT.float32)
            cC = selb.tile([128, 512], DT.float32)
            t1 = selb.tile([128, 512], DT.float32)
            t2 = selb.tile([128, 512], DT.float32)
            _iotas.append(nc.gpsimd.iota(rP[:], pattern=[[0, 1]], channel_multiplier=1,
                           allow_small_or_imprecise_dtypes=True))
            _iotas.append(nc.gpsimd.iota(cC[:], pattern=[[1, 512]], channel_multiplier=0,
                           allow_small_or_imprecise_dtypes=True))
            # j = r mod 32 ; qi_r = (r - j)/32
            j32 = selb.tile([128, 1], DT.float32)
            qir = selb.tile([128, 1], DT.float32)
            jt = selb.tile([128, 1], DT.float32)
            nc.vector.tensor_scalar(out=qir[:], in0=rP[:], scalar1=31.5, scalar2=None, op0=ALU.is_gt)
            nc.vector.tensor_scalar(out=jt[:], in0=rP[:], scalar1=63.5, scalar2=None, op0=ALU.is_gt)
            nc.vector.tensor_add(qir[:], qir[:], jt[:])
            nc.vector.tensor_scalar(out=jt[:], in0=rP[:], scalar1=95.5, scalar2=None, op0=ALU.is_gt)
            nc.vector.tensor_add(qir[:], qir[:], jt[:])
            nc.vector.tensor_scalar(out=j32[:], in0=qir[:], scalar1=-32.0, scalar2=None, op0=ALU.mult)
            nc.vector.tensor_add(j32[:], j32[:], rP[:])
            # cond1: floor(c/128) == qi_r  -> |c/128 - qi_r - frac| via mod
            t3 = selb.tile([128, 512], DT.float32)
            nc.vector.tensor_scalar(out=t2[:], in0=cC[:], scalar1=127.5, scalar2=None, op0=ALU.is_gt)
            nc.vector.tensor_scalar(out=t3[:], in0=cC[:], scalar1=255.5, scalar2=None, op0=ALU.is_gt)
            nc.vector.tensor_add(t2[:], t2[:], t3[:])
            nc.vector.tensor_scalar(out=t3[:], in0=cC[:], scalar1=383.5, scalar2=None, op0=ALU.is_gt)
            nc.vector.tensor_add(t2[:], t2[:], t3[:])   # floor(c/128)
            nc.vector.tensor_scalar(out=t1[:], in0=t2[:], scalar1=-128.0, scalar2=None, op0=ALU.mult)
            nc.vector.tensor_add(t1[:], t1[:], cC[:])   # c mod 128
            nc.vector.tensor_scalar(out=t2[:], in0=t2[:], scalar1=qir[:], scalar2=None,
                                    op0=ALU.is_equal)
            # cond2: floor((c mod 128)/64) == floor(j/9)  (j<18 -> floor(j/9) in {0,1})
            nc.vector.tensor_scalar(out=t1[:], in0=t1[:], scalar1=63.5, scalar2=None,
                                    op0=ALU.is_gt)             # h(c)
            hj = selb.tile([128, 1], DT.float32)
            nc.vector.tensor_scalar(out=hj[:], in0=j32[:], scalar1=8.5, scalar2=None,
                                    op0=ALU.is_gt)             # j>=9
            nc.vector.tensor_scalar(out=t1[:], in0=t1[:], scalar1=hj[:], scalar2=None,
                                    op0=ALU.is_equal)
            nc.vector.tensor_mul(t2[:], t2[:], t1[:])
            # cond3: j < 18
            j18 = selb.tile([128, 1], DT.float32)
            nc.vector.tensor_scalar(out=j18[:], in0=j32[:], scalar1=31.5, scalar2=None,
                                    op0=ALU.is_lt)
            nc.vector.tensor_scalar(out=t2[:], in0=t2[:], scalar1=j18[:], scalar2=None,
                                    op0=ALU.mult)
            # per-k selectivity: jk = j32 - 9*hj ; sel_k = t2 * (jk == k)
            jkk = selb.tile([128, 1], DT.float32)
            nc.vector.tensor_scalar(out=jkk[:], in0=hj[:], scalar1=-9.0, scalar2=None,
                                    op0=ALU.mult)
            nc.vector.tensor_add(jkk[:], jkk[:], j32[:])
            tk = selb.tile([128, 1], DT.float32)
            for k in range(KF):
                nc.vector.tensor_scalar(out=tk[:], in0=jkk[:], scalar1=float(k),
                                        scalar2=None, op0=ALU.is_equal)
                nc.vector.tensor_scalar(out=sel[:, k * 512 : (k + 1) * 512],
                                        in0=t2[:], scalar1=tk[:, 0:1],
                                        scalar2=None, op0=ALU.mult)

        # wdup + b_dc
        nc.vector.memset(wdup[:], 0.0)
        wdc_v = wdc_ext[:].rearrange("o c kh kw -> c o (kh kw)")
        for k in range(KF):
            for h in range(2):
                nc.gpsimd.dma_start(
                    out=wdup[h * 64 : h * 64 + 64,
                             k * 128 + h * 64 : k * 128 + h * 64 + 64],
                    in_=wdc_v[:, :, k : k + 1].rearrange("c a d -> c (a d)"),
                )
        for h in range(2):
            nc.sync.dma_start(
                out=bdc_t[h * 64 : h * 64 + 64, 0:1],
                in_=bdc_ext[:].rearrange("(o one) -> o one", one=1),
            )

        # ======== quad table build (needs x_sb; before xpool closes)
        nc.gpsimd.memset(qtab[:], 0.0)
        q4 = qtab[:].rearrange("p (blk q) -> p blk q", q=4)
        for a in range(2):
            for b in range(2):
                blk0 = (a * 2 + b) * (RY_N * RX_N)
                for qy in range(2):
                    for qx in range(2):
                        ry_cnt = min((75 - a - qy) // 2 + 1, RY_N)
                        rx0 = 1 if (b + qx) == 0 else 0
                        rx1 = min(RX_N - 1, (130 - b - qx) // 2)
                        rx_cnt = rx1 - rx0 + 1
                        c0 = 2 * rx0 + b + qx - 1
                        src = x3()[:, a + qy : a + qy + 2 * (ry_cnt - 1) + 1 : 2,
                                   c0 : c0 + 2 * (rx_cnt - 1) + 1 : 2]
                        dst3 = q4[:, blk0 + rx0 : blk0 + rx0 + (ry_cnt - 1) * RX_N + rx_cnt,
                                  qy * 2 + qx : qy * 2 + qx + 1]
                        dst = bass.AP(
                            dst3.tensor, dst3.offset,
                            [dst3.ap[0], [RX_N * 4, ry_cnt], [4, rx_cnt]],
                        )
                        nc.scalar.activation(out=dst, in_=src, func=AF.Copy)

        xpool.__exit__(None, None, None)

        # ======== phase 4+5: gathers (k-pairs) + modulate + matmul
        with (
            tc.tile_pool(name="g", bufs=2) as gpool,
            tc.tile_pool(name="h", bufs=2) as hpool,
            tc.tile_pool(name="o", bufs=2) as opool,
            tc.tile_pool(name="mp", bufs=4, space="PSUM") as mpsum,
            tc.tile_pool(name="op", bufs=1, space="PSUM") as opsum,
        ):
            _lib = nc.gpsimd.load_library(library_config.ap_gather)
            for _io in _iotas:
                tile.add_dep_helper(_lib.ins, _io.ins, reason="lib load after iotas")
            qtab_f32 = qtab[:].bitcast(DT.float32)
            outv = out_ext[:].rearrange("o h w -> o (h w)")

            KP = [(0, 1), (2, 3), (4, 5), (6, 7), (8,)]
            for cb in range(NPIX // GCHUNK):
                po = opsum.tile([128, GCHUNK], DT.float32, tag="po")
                for kp in KP:
                    nk = len(kp)
                    g = gpool.tile([128, 2 * GCHUNK * 2], DT.float32, tag="g")
                    idx_sl = idxt[:, (cb * KF + kp[0]) * 128 :
                                  (cb * KF + kp[0] + nk) * 128]
                    _ga = nc.gpsimd.ap_gather(
                        g[:, 0 : nk * GCHUNK * 2], qtab_f32, idx_sl,
                        channels=128, num_elems=NBLK, d=2, num_idxs=nk * GCHUNK,
                    )
                    tile.add_dep_helper(_ga.ins, _lib.ins, reason="gather after lib load")
                    gb = g[:].bitcast(DT.bfloat16).rearrange(
                        "p (n q) -> p n q", q=4
                    )
                    for ki, k in enumerate(kp):
                        for sub in range(GCHUNK // SUB):
                            col0 = cb * GCHUNK + sub * SUB
                            hts = []
                            for qi in range(4):
                                mq = mpsum.tile([128, SUB], DT.float32, tag="mq")
                                nc.tensor.matmul(
                                    out=mq[:],
                                    lhsT=sel[:, k * 512 + qi * 128 : k * 512 + (qi + 1) * 128],
                                    rhs=wq[:, col0 : col0 + SUB],
                                    start=True, stop=True,
                                )
                                ht = hpool.tile([128, SUB], DT.bfloat16, tag=f"ht{qi}")
                                gq = gb[:, ki * GCHUNK + sub * SUB :
                                        ki * GCHUNK + (sub + 1) * SUB,
                                        qi : qi + 1].rearrange("p n one -> p (n one)")
                                nc.vector.tensor_mul(ht[:], mq[:], gq)
                                hts.append(ht)
                            for qi in range(4):
                                nc.tensor.matmul(
                                    out=po[:, sub * SUB : (sub + 1) * SUB],
                                    lhsT=wdup[:, k * 128 : (k + 1) * 128],
                                    rhs=hts[qi][:],
                                    start=(k == 0 and qi == 0),
                                    stop=(k == KF - 1 and qi == 3),
                                )
                ot = opool.tile([128, GCHUNK], DT.float32, tag="ot")
                nc.vector.tensor_scalar(
                    out=ot[:], in0=po[:], scalar1=bdc_t[:, 0:1],
                    scalar2=None, op0=ALU.add,
                )
                for h in range(2):
                    nc.sync.dma_start(
                        out=outv[:, h * NPIX + cb * GCHUNK:
                                 h * NPIX + (cb + 1) * GCHUNK],
                        in_=ot[h * 64 : h * 64 + 64, :],
                    )


def _build_nc():
    _install_compat()
    nc = bass.Bass()
    x_ext = nc.declare_dram_parameter("x", [C, H, W], DT.float32, isOutput=False)
    wom_ext = nc.declare_dram_parameter("w_om", [3 * KF, C, K, K], DT.float32, isOutput=False)
    bom_ext = nc.declare_dram_parameter("b_om", [3 * KF], DT.float32, isOutput=False)
    wdc_ext = nc.declare_dram_parameter("w_dc", [O, C, K, K], DT.float32, isOutput=False)
    bdc_ext = nc.declare_dram_parameter("b_dc", [O], DT.float32, isOutput=False)
    out_ext = nc.declare_dram_parameter("out", [O, H, W], DT.float32, isOutput=True)
    with tile.TileContext(nc) as tc:
        _emit(nc, tc, x_ext, wom_ext, bom_ext, wdc_ext, bdc_ext, out_ext)
    lower_extended_insts(nc)
    return nc


_NC_CACHE = None


def kernel(**inputs):
    global _NC_CACHE
    x = np.ascontiguousarray(inputs["x"], dtype=np.float32)
    w_om = np.ascontiguousarray(inputs["w_om"], dtype=np.float32)
    b_om = np.ascontiguousarray(inputs["b_om"], dtype=np.float32)
    w_dc = np.ascontiguousarray(inputs["w_dc"], dtype=np.float32)
    b_dc = np.ascontiguousarray(inputs["b_dc"], dtype=np.float32)

    if _NC_CACHE is None:
        _NC_CACHE = _build_nc()
    nc = _NC_CACHE

    in_maps = [
        {"x": x[i], "w_om": w_om, "b_om": b_om, "w_dc": w_dc, "b_dc": b_dc}
        for i in range(NCORES)
    ]
    res = run_bass_kernel_spmd(nc, in_maps, core_ids=list(range(NCORES)))
    return np.stack(
        [np.asarray(res.results[i]["out"]) for i in range(NCORES)]
    ).astype(np.float32)

